# revision 13
# baseline (speedup 1.0000x reference)
"""Trainium2 Bass kernel for nn_EncoderLayer (dense transformer encoder layer).

Sharding: data-parallel over batch b across 8 NeuronCores (one batch element
per core, weights replicated, no collectives).

Per-core plan:
  - temporal attention per k-slice (L=256 tokens), token-major tiles
    [128 part = l%128, a = l//128, d]
  - tem_out round-trips through DRAM to reorder tokens into "type order"
    t' = l*32 + k; a type tile of 128 tokens = 4 complete l-slices
    (partition p = 32*(l%4) + k)
  - type attention batches 4 l-slices per [128,128] logits tile with a
    block-diagonal mask; fully-masked rows recover uniform 1/K via the
    (S+C)*M01 trick plus a 4x row rescale
  - FFN on 512-token batches in type order; final LN + scatter-store back
    to temporal token order

Matmul operands are kept in bf16 (PE runs 1 cycle/row); PSUM accumulation
and the softmax logits stay fp32.
"""

import sys

for _p in ("/opt/trn_rl_repo",):
    if _p not in sys.path:
        sys.path.append(_p)

import numpy as np
import ml_dtypes

import concourse.bacc as bacc
import concourse.mybir as mybir
import concourse.tile as tile
from concourse.bass import ts

F32 = mybir.dt.float32
BF16 = mybir.dt.bfloat16
I32 = mybir.dt.int32
AF = mybir.ActivationFunctionType
ALU = mybir.AluOpType

B, K, L, D = 8, 32, 256, 256
NH, DK, DV, DI = 4, 64, 64, 1024
P = 128
C_MASK = 64.0


def build_nc():
    nc = bacc.Bacc("TRN2", target_bir_lowering=False, debug=False)

    x = nc.dram_tensor("x", (K, L, D), F32, kind="ExternalInput")
    mask = nc.dram_tensor("mask", (K, L), I32, kind="ExternalInput")
    wq_t = nc.dram_tensor("Wq_t", (D, NH * DK), F32, kind="ExternalInput")
    wk_t = nc.dram_tensor("Wk_t", (D, NH * DK), F32, kind="ExternalInput")
    wv_t = nc.dram_tensor("Wv_t", (D, NH * DV), F32, kind="ExternalInput")
    fcw_t = nc.dram_tensor("fcw_t", (NH * DV, D), F32, kind="ExternalInput")
    fcb_t = nc.dram_tensor("fcb_t", (D,), F32, kind="ExternalInput")
    lng_t = nc.dram_tensor("lng_t", (D,), F32, kind="ExternalInput")
    lnb_t = nc.dram_tensor("lnb_t", (D,), F32, kind="ExternalInput")
    wq_y = nc.dram_tensor("Wq_y", (D, NH * DK), F32, kind="ExternalInput")
    wk_y = nc.dram_tensor("Wk_y", (D, NH * DK), F32, kind="ExternalInput")
    wv_y = nc.dram_tensor("Wv_y", (D, NH * DV), F32, kind="ExternalInput")
    fcw_y = nc.dram_tensor("fcw_y", (NH * DV, D), F32, kind="ExternalInput")
    fcb_y = nc.dram_tensor("fcb_y", (D,), F32, kind="ExternalInput")
    lng_y = nc.dram_tensor("lng_y", (D,), F32, kind="ExternalInput")
    lnb_y = nc.dram_tensor("lnb_y", (D,), F32, kind="ExternalInput")
    w1 = nc.dram_tensor("w1", (D, DI), F32, kind="ExternalInput")
    b1 = nc.dram_tensor("b1", (DI,), F32, kind="ExternalInput")
    w2 = nc.dram_tensor("w2", (DI, D), F32, kind="ExternalInput")
    b2 = nc.dram_tensor("b2", (D,), F32, kind="ExternalInput")
    lng = nc.dram_tensor("lng", (D,), F32, kind="ExternalInput")
    lnb = nc.dram_tensor("lnb", (D,), F32, kind="ExternalInput")

    out = nc.dram_tensor("out", (K, L, D), F32, kind="ExternalOutput")
    attn_tem = nc.dram_tensor("attn_tem", (K, NH, L, L), F32, kind="ExternalOutput")
    attn_type = nc.dram_tensor("attn_type", (L, NH, K, K), F32, kind="ExternalOutput")

    tem_d = nc.dram_tensor("tem_d", (K, L, D), F32, kind="Internal")
    enc_d = nc.dram_tensor("enc_d", (K * L, D), F32, kind="Internal")
    mrow_d = nc.dram_tensor("mrow_d", (K * L,), BF16, kind="Internal")
    mtyp_d = nc.dram_tensor("mtyp_d", (K * L,), BF16, kind="Internal")

    ident_d = nc.inline_tensor(np.eye(P, dtype=np.float32), name="ident_c")
    identb_d = nc.inline_tensor(np.eye(P, dtype=ml_dtypes.bfloat16), name="identb_c")
    onesb_d = nc.inline_tensor(np.ones((1, P), dtype=ml_dtypes.bfloat16), name="onesb_c")
    bdiag_np = np.kron(np.eye(4), np.ones((K, K))).astype(ml_dtypes.bfloat16)
    bdiag_d = nc.inline_tensor(bdiag_np, name="bdiag_c")

    with tile.TileContext(nc) as tc:
        cp = tc.alloc_tile_pool(name="const", bufs=1)
        wp = tc.alloc_tile_pool(name="weights", bufs=1)
        psp = tc.alloc_tile_pool(name="psp", bufs=8, space="PSUM")
        setp = tc.alloc_tile_pool(name="setup", bufs=2)

        ident = cp.tile([P, P], F32)
        nc.sync.dma_start(ident[:], ident_d[:])
        identb = cp.tile([P, P], BF16)
        nc.sync.dma_start(identb[:], identb_d[:])
        onesb = cp.tile([1, P], BF16)
        nc.sync.dma_start(onesb[:], onesb_d[:])
        bdiag = cp.tile([P, P], BF16)
        nc.sync.dma_start(bdiag[:], bdiag_d[:])
        negC = cp.tile([P, 1], F32)
        nc.vector.memset(negC[:], -C_MASK)

        # ---- mask preprocessing ----
        msk_i = setp.tile([K, L], I32)
        nc.sync.dma_start(msk_i[:], mask[:])
        msk_f = setp.tile([K, L], F32)
        nc.vector.tensor_copy(msk_f[:], msk_i[:])
        nc.vector.tensor_scalar(msk_f[:], msk_f[:], 0.0, None, op0=ALU.not_equal)
        msk_b = setp.tile([K, L], BF16)
        nc.vector.tensor_copy(msk_b[:], msk_f[:])
        nc.sync.dma_start(mrow_d.rearrange("(k l) -> k l", k=K), msk_b[:])
        mT = setp.tile([P, 2, K], BF16)
        for a in range(2):
            tps = psp.tile([P, K], F32, tag="ps")
            nc.tensor.transpose(tps[:], msk_f[:, ts(a, P)], ident[0:K, 0:K])
            nc.scalar.copy(mT[:, a, :], tps[:])
        nc.sync.dma_start(mtyp_d.rearrange("(a r k) -> r a k", a=2, r=P), mT[:])

        # ---- weights: load f32, fold shared-LN gain, convert to bf16 ----
        lng_pp = wp.tile([P, 2], F32)
        nc.sync.dma_start(lng_pp[:], lng.rearrange("(c p) -> p c", p=P))
        lnb_ppb = wp.tile([P, 2], BF16)
        lnb_pp = setp.tile([P, 2], F32, tag="lnbpp")
        nc.sync.dma_start(lnb_pp[:], lnb.rearrange("(c p) -> p c", p=P))
        nc.vector.tensor_copy(lnb_ppb[:], lnb_pp[:])

        def load_wb(dram, cdim, odim, name, fold, scale=1.0):
            stage = setp.tile([P, cdim // P, odim], F32, tag="wstage")
            nc.sync.dma_start(stage[:], dram.rearrange("(c p) o -> p c o", p=P))
            t = wp.tile([P, cdim // P, odim], BF16, name=name)
            for c in range(cdim // P):
                if fold:
                    nc.vector.tensor_scalar(
                        stage[:, c, :], stage[:, c, :], lng_pp[:, c : c + 1],
                        float(scale), op0=ALU.mult, op1=ALU.mult,
                    )
                nc.gpsimd.tensor_copy(t[:, c, :], stage[:, c, :])
            return t

        wq_ts = load_wb(wq_t, D, NH * DK, "wq_ts", True, 0.125)
        wk_ts = load_wb(wk_t, D, NH * DK, "wk_ts", True)
        wv_ts = load_wb(wv_t, D, NH * DV, "wv_ts", True)
        fcw_ts = load_wb(fcw_t, NH * DV, D, "fcw_ts", False)
        wq_ys = load_wb(wq_y, D, NH * DK, "wq_ys", True, 0.125)
        wk_ys = load_wb(wk_y, D, NH * DK, "wk_ys", True)
        wv_ys = load_wb(wv_y, D, NH * DV, "wv_ys", True)
        fcw_ys = load_wb(fcw_y, NH * DV, D, "fcw_ys", False)
        w1s = load_wb(w1, D, DI, "w1s", True)
        w2s = load_wb(w2, DI, D, "w2s", False)

        # folded-LN bias per output tile: bW[o] = sum_d lnb[d] * W'[d, o]
        def fold_bias(wt, nout, name):
            bt = wp.tile([P, nout // P, 1], F32, name=name)
            for ot in range(nout // P):
                ps = psp.tile([P, 1], F32, tag="ps")
                for c in range(2):
                    nc.tensor.matmul(ps[:], wt[:, c, ts(ot, P)],
                                     lnb_ppb[:, c : c + 1],
                                     start=(c == 0), stop=(c == 1))
                nc.scalar.copy(bt[:, ot, :], ps[:])
            return bt

        bwq_t = fold_bias(wq_ts, NH * DK, "bwq_t")
        bwk_t = fold_bias(wk_ts, NH * DK, "bwk_t")
        bwq_y = fold_bias(wq_ys, NH * DK, "bwq_y")
        bwk_y = fold_bias(wk_ys, NH * DK, "bwk_y")
        bw1 = fold_bias(w1s, DI, "bw1")

        b1_pp = setp.tile([P, DI // P], F32, tag="b1pp")
        nc.sync.dma_start(b1_pp[:], b1.rearrange("(c p) -> p c", p=P))
        b1f = wp.tile([P, DI // P, 1], F32)
        for c in range(DI // P):
            nc.vector.tensor_tensor(
                b1f[:, c, :], b1_pp[:, c : c + 1], bw1[:, c, :], op=ALU.add
            )

        # broadcast-row tiles [128, D] for per-column vectors
        def bcast_row(dram_vec, name):
            row = setp.tile([1, D], F32, tag="bcrow")
            nc.sync.dma_start(row[:], dram_vec[None, :])
            t = wp.tile([P, D], F32, name=name)
            nc.gpsimd.partition_broadcast(t[:], row[:])
            return t

        fcb_t_b = bcast_row(fcb_t, "fcb_t_b")
        fcb_y_b = bcast_row(fcb_y, "fcb_y_b")
        b2_b = bcast_row(b2, "b2_b")
        gt_b = bcast_row(lng_t, "gt_b")
        bt_b = bcast_row(lnb_t, "bt_b")
        gy_b = bcast_row(lng_y, "gy_b")
        by_b = bcast_row(lnb_y, "by_b")
        gf_b = bcast_row(lng, "gf_b")
        bf_b = bcast_row(lnb, "bf_b")

        # V-bias broadcast rows: bWv[o] broadcast to [128, D]
        def fold_bias_row(wt, name):
            bcol = fold_bias(wt, D, name + "_col")
            rowp = setp.tile([1, D], F32, tag="fbrow")
            for c in range(2):
                tps = psp.tile([1, P], F32, tag="ps")
                nc.tensor.transpose(tps[:], bcol[:, c, :], ident[:])
                nc.scalar.copy(rowp[:, ts(c, P)], tps[:])
            t = wp.tile([P, D], F32, name=name)
            nc.gpsimd.partition_broadcast(t[:], rowp[:])
            return t

        bwv_t_b = fold_bias_row(wv_ts, "bwv_t_b")
        bwv_y_b = fold_bias_row(wv_ys, "bwv_y_b")

        def batched_rstd(pool, var_ap, eps, n, tag):
            veps = pool.tile([P, n], F32, tag=tag + "ve")
            nc.vector.tensor_scalar(veps[:], var_ap, float(eps), None, op0=ALU.add)
            lnv = pool.tile([P, n], F32, tag=tag + "ln")
            nc.scalar.activation(lnv[:], veps[:], AF.Ln)
            rstd = pool.tile([P, n], F32, tag=tag + "rs")
            nc.scalar.activation(rstd[:], lnv[:], AF.Exp, scale=-0.5)
            return rstd

        def ln_norm(pool, src_ap_list, eps, tag, out_tiles):
            """LayerNorm normalize (no gain/bias) for [128, D] APs sharing one
            batched rstd computation."""
            n = len(src_ap_list)
            mv = pool.tile([P, n, 2], F32, tag=tag + "mv")
            for i, ap in enumerate(src_ap_list):
                st = pool.tile([P, 6], F32, tag=tag + "st")
                nc.vector.bn_stats(st[:], ap)
                nc.vector.bn_aggr(mv[:, i, :], st[:])
            rstd = batched_rstd(pool, mv[:, :, 1], eps, n, tag)
            negmr = pool.tile([P, n], F32, tag=tag + "nm")
            nc.vector.scalar_tensor_tensor(
                negmr[:], mv[:, :, 0], -1.0, rstd[:], op0=ALU.mult, op1=ALU.mult
            )
            for i, ap in enumerate(src_ap_list):
                nc.scalar.activation(
                    out_tiles[i], ap, AF.Identity,
                    bias=negmr[:, i : i + 1], scale=rstd[:, i : i + 1],
                )

        # ================= STAGE 1: temporal attention =================
        setp.release()
        s1 = tc.alloc_tile_pool(name="s1", bufs=2)
        s1s = tc.alloc_tile_pool(name="s1_small", bufs=3)

        for k in range(K):
            xk = s1.tile([P, 2, D], F32, tag="xk")
            nc.sync.dma_start(xk[:], x[k].rearrange("(a p) d -> p a d", p=P))

            xh = s1.tile([P, 2, D], BF16, tag="xh")
            ln_norm(s1s, [xk[:, 0, :], xk[:, 1, :]], 1e-6, "ln1",
                    [xh[:, 0, :], xh[:, 1, :]])

            tp = psp.tile([P, 2, 2 * P], BF16, tag="ps")
            for a in range(2):
                for dc in range(2):
                    nc.tensor.transpose(tp[:, dc, ts(a, P)], xh[:, a, ts(dc, P)],
                                        identb[:])
            xhT = s1.tile([P, 2, 2 * P], BF16, tag="xhT")
            nc.scalar.copy(xhT[:], tp[:])

            qT = s1.tile([P, 2, 2 * P], BF16, tag="qT")
            kT = s1.tile([P, 2, 2 * P], BF16, tag="kT")
            for wt, bt, dst in ((wq_ts, bwq_t, qT), (wk_ts, bwk_t, kT)):
                for ot in range(2):
                    ps = psp.tile([P, 2 * P], F32, tag="ps")
                    for c in range(2):
                        nc.tensor.matmul(ps[:], wt[:, c, ts(ot, P)], xhT[:, c, :],
                                         start=(c == 0), stop=(c == 1))
                    nc.scalar.activation(
                        dst[:, ot, :], ps[:], AF.Identity, bias=bt[:, ot, :]
                    )
            v_sb = s1.tile([P, 2, D], BF16, tag="v_sb")
            for a in range(2):
                ps = psp.tile([P, D], F32, tag="ps")
                for c in range(2):
                    nc.tensor.matmul(ps[:], xhT[:, c, ts(a, P)], wv_ts[:, c, :],
                                     start=(c == 0), stop=(c == 1))
                nc.vector.scalar_tensor_tensor(
                    v_sb[:, a, :], ps[:], 1.0, bwv_t_b[:], op0=ALU.mult, op1=ALU.add
                )

            mrow_k = s1s.tile([1, L], BF16, tag="mrowk")
            nc.sync.dma_start(mrow_k[:], mrow_d[None, k * L : (k + 1) * L])
            mps = psp.tile([P, 2, L], F32, tag="ps")
            for lt in range(2):
                nc.tensor.matmul(mps[:, lt, :], mrow_k[:, ts(lt, P)], mrow_k[:],
                                 start=True, stop=True)
            m01 = s1.tile([P, 2, L], BF16, tag="m01")
            nc.scalar.copy(m01[:], mps[:])

            avT = s1.tile([P, 2, 2 * P], BF16, tag="avT")
            avp = psp.tile([P, 2, L], F32, tag="ps")
            for h in range(NH):
                hp = (h % 2) * DK
                q_h = qT[hp : hp + DK, h // 2, :]
                k_h = kT[hp : hp + DK, h // 2, :]
                sps = psp.tile([P, 2, L], F32, tag="ps")
                for lt in range(2):
                    nc.tensor.matmul(sps[:, lt, :], q_h[:, ts(lt, P)], k_h[:],
                                     start=True, stop=True)
                tt = s1.tile([P, 2, L], F32, tag="tt")
                nc.vector.scalar_tensor_tensor(
                    tt[:], sps[:], C_MASK, m01[:], op0=ALU.add, op1=ALU.mult
                )
                atp = psp.tile([P, 2, L], F32, tag="ps")
                den = s1s.tile([P, 2], F32, tag="den")
                rec = s1s.tile([P, 2], F32, tag="rec")
                for lt in range(2):
                    p_sb = s1.tile([P, L], BF16, tag="p_sb")
                    nc.scalar.activation(
                        p_sb[:], tt[:, lt, :], AF.Exp, bias=negC[:],
                        accum_out=den[:, lt : lt + 1],
                    )
                    nc.vector.reciprocal(rec[:, lt : lt + 1], den[:, lt : lt + 1])
                    attn = s1.tile([P, L], F32, tag="attn")
                    nc.vector.tensor_scalar(
                        attn[:], p_sb[:], rec[:, lt : lt + 1], None, op0=ALU.mult
                    )
                    nc.sync.dma_start(attn_tem[k, h, ts(lt, P), :], attn[:])
                    for mt in range(2):
                        nc.tensor.transpose(atp[:, mt, ts(lt, P)],
                                            attn[:, ts(mt, P)], ident[:])
                atT = s1.tile([P, 2, L], BF16, tag="atT")
                nc.scalar.copy(atT[:], atp[:])
                for mt in range(2):
                    nc.tensor.matmul(avp[hp : hp + DV, h // 2, :],
                                     v_sb[:, mt, h * DV : (h + 1) * DV],
                                     atT[:, mt, :],
                                     start=(mt == 0), stop=(mt == 1))
            nc.scalar.copy(avT[:], avp[:])

            tem_sb = s1.tile([P, 2, D], F32, tag="tem_sb")
            ob = s1.tile([P, 2, D], F32, tag="ob")
            for a in range(2):
                ops = psp.tile([P, D], F32, tag="ps")
                for c in range(2):
                    nc.tensor.matmul(ops[:], avT[:, c, ts(a, P)], fcw_ts[:, c, :],
                                     start=(c == 0), stop=(c == 1))
                nc.vector.scalar_tensor_tensor(
                    ob[:, a, :], ops[:], 1.0, fcb_t_b[:], op0=ALU.mult, op1=ALU.add
                )
            xht = s1.tile([P, 2, D], F32, tag="xht")
            ln_norm(s1s, [ob[:, 0, :], ob[:, 1, :]], 1e-5, "lnt",
                    [xht[:, 0, :], xht[:, 1, :]])
            for a in range(2):
                u = s1.tile([P, D], F32, tag="u")
                nc.gpsimd.tensor_tensor(u[:], xk[:, a, :], bt_b[:], op=ALU.add)
                t1 = s1.tile([P, D], F32, tag="t1")
                nc.vector.tensor_tensor(t1[:], xht[:, a, :], gt_b[:], op=ALU.mult)
                nc.gpsimd.tensor_tensor(tem_sb[:, a, :], t1[:], u[:], op=ALU.add)
            nc.sync.dma_start(
                tem_d[k].rearrange("(a p) d -> p a d", p=P), tem_sb[:]
            )

        # ================= STAGE 2: type attention =================
        for gp in range(K):  # 32 pairs of type tiles
            tin = s1.tile([P, 2, D], F32, tag="tin")
            for i in range(4):
                nc.sync.dma_start(
                    tin[K * i : K * (i + 1), :, :],
                    tem_d[:, 8 * gp + i : 8 * gp + i + 5 : 4, :],
                )

            xh2 = s1.tile([P, 2, D], BF16, tag="xh")
            ln_norm(s1s, [tin[:, 0, :], tin[:, 1, :]], 1e-6, "ln2",
                    [xh2[:, 0, :], xh2[:, 1, :]])
            tp = psp.tile([P, 2, 2 * P], BF16, tag="ps")
            for t2 in range(2):
                for dc in range(2):
                    nc.tensor.transpose(tp[:, dc, ts(t2, P)], xh2[:, t2, ts(dc, P)],
                                        identb[:])
            xh2T = s1.tile([P, 2, 2 * P], BF16, tag="xhT")
            nc.scalar.copy(xh2T[:], tp[:])

            q2T = s1.tile([P, 2, 2 * P], BF16, tag="qT")
            k2T = s1.tile([P, 2, 2 * P], BF16, tag="kT")
            for wt, bt, dst in ((wq_ys, bwq_y, q2T), (wk_ys, bwk_y, k2T)):
                for ot in range(2):
                    ps = psp.tile([P, 2 * P], F32, tag="ps")
                    for c in range(2):
                        nc.tensor.matmul(ps[:], wt[:, c, ts(ot, P)], xh2T[:, c, :],
                                         start=(c == 0), stop=(c == 1))
                    nc.scalar.activation(
                        dst[:, ot, :], ps[:], AF.Identity, bias=bt[:, ot, :]
                    )
            v2 = s1.tile([P, 2, D], BF16, tag="v_sb")
            for t2 in range(2):
                ps = psp.tile([P, D], F32, tag="ps")
                for c in range(2):
                    nc.tensor.matmul(ps[:], xh2T[:, c, ts(t2, P)], wv_ys[:, c, :],
                                     start=(c == 0), stop=(c == 1))
                nc.vector.scalar_tensor_tensor(
                    v2[:, t2, :], ps[:], 1.0, bwv_y_b[:], op0=ALU.mult, op1=ALU.add
                )

            enc2 = s1.tile([P, 2, D], F32, tag="enc2")
            for t2 in range(2):
                g = 2 * gp + t2
                vrow_g = s1s.tile([1, P], BF16, tag="vrowg")
                nc.sync.dma_start(vrow_g[:], mtyp_d[None, g * P : (g + 1) * P])
                mps = psp.tile([P, P], F32, tag="ps")
                nc.tensor.matmul(mps[:], vrow_g[:], vrow_g[:], start=True, stop=True)
                m01g = s1.tile([P, P], BF16, tag="m01g")
                nc.vector.tensor_tensor(m01g[:], mps[:], bdiag[:], op=ALU.mult)
                vcp = psp.tile([P, 1], F32, tag="ps")
                nc.tensor.matmul(vcp[:], vrow_g[:], onesb[:, 0:1],
                                 start=True, stop=True)
                rs = s1s.tile([P, 1], F32, tag="rs")
                nc.vector.tensor_scalar(
                    rs[:], vcp[:], -3.0, 4.0, op0=ALU.mult, op1=ALU.add
                )

                attn2_all = s1.tile([P, NH, P], F32, tag="attn2_all")
                av2T = s1.tile([P, 2, P], BF16, tag="av2T")
                av2p = psp.tile([P, 2, P], F32, tag="ps")
                for h in range(NH):
                    hp = (h % 2) * DK
                    sps = psp.tile([P, P], F32, tag="ps")
                    nc.tensor.matmul(sps[:], q2T[hp : hp + DK, h // 2, ts(t2, P)],
                                     k2T[hp : hp + DK, h // 2, ts(t2, P)],
                                     start=True, stop=True)
                    tt = s1.tile([P, P], F32, tag="tt2")
                    nc.vector.scalar_tensor_tensor(
                        tt[:], sps[:], C_MASK, m01g[:], op0=ALU.add, op1=ALU.mult
                    )
                    den = s1s.tile([P, 1], F32, tag="den2")
                    p_sb = s1.tile([P, P], BF16, tag="p_sb2")
                    nc.scalar.activation(
                        p_sb[:], tt[:], AF.Exp, bias=negC[:], accum_out=den[:]
                    )
                    rec = s1s.tile([P, 1], F32, tag="rec2a")
                    nc.vector.reciprocal(rec[:], den[:])
                    rec2 = s1s.tile([P, 1], F32, tag="rec2b")
                    nc.vector.tensor_tensor(rec2[:], rec[:], rs[:], op=ALU.mult)
                    nc.vector.scalar_tensor_tensor(
                        attn2_all[:, h, :], p_sb[:], rec2[:], bdiag[:],
                        op0=ALU.mult, op1=ALU.mult,
                    )
                    atp = psp.tile([P, P], F32, tag="ps")
                    nc.tensor.transpose(atp[:], attn2_all[:, h, :], ident[:])
                    at2 = s1.tile([P, P], BF16, tag="at2s")
                    nc.scalar.copy(at2[:], atp[:])
                    nc.tensor.matmul(av2p[hp : hp + DV, h // 2, :],
                                     v2[:, t2, h * DV : (h + 1) * DV], at2[:],
                                     start=True, stop=True)
                for i in range(4):
                    nc.sync.dma_start(
                        attn_type[4 * g + i].rearrange("h k1 k2 -> k1 h k2"),
                        attn2_all[K * i : K * (i + 1), :, K * i : K * (i + 1)],
                    )
                nc.scalar.copy(av2T[:], av2p[:])

                ops = psp.tile([P, D], F32, tag="ps")
                for c in range(2):
                    nc.tensor.matmul(ops[:], av2T[:, c, :], fcw_ys[:, c, :],
                                     start=(c == 0), stop=(c == 1))
                ob2 = s1.tile([P, D], F32, tag="ob2")
                nc.vector.scalar_tensor_tensor(
                    ob2[:], ops[:], 1.0, fcb_y_b[:], op0=ALU.mult, op1=ALU.add
                )
                xhy = s1.tile([P, D], F32, tag="xht")
                ln_norm(s1s, [ob2[:]], 1e-5, "lny", [xhy[:]])
                u = s1.tile([P, D], F32, tag="u")
                nc.gpsimd.tensor_tensor(u[:], tin[:, t2, :], by_b[:], op=ALU.add)
                t1 = s1.tile([P, D], F32, tag="t1")
                nc.vector.tensor_tensor(t1[:], xhy[:], gy_b[:], op=ALU.mult)
                nc.gpsimd.tensor_tensor(enc2[:, t2, :], t1[:], u[:], op=ALU.add)
            nc.sync.dma_start(
                enc_d.rearrange("(gg p) d -> gg p d", p=P)[2 * gp], enc2[:, 0, :]
            )
            nc.sync.dma_start(
                enc_d.rearrange("(gg p) d -> gg p d", p=P)[2 * gp + 1], enc2[:, 1, :]
            )

        # ================= STAGE 3: FFN + final LN =================
        s1s.release()
        s1.release()
        f1 = tc.alloc_tile_pool(name="f1", bufs=2)
        f1s = tc.alloc_tile_pool(name="f1_small", bufs=3)
        enc_b = enc_d.rearrange("(b t p) d -> b p t d", t=4, p=P)
        for bb in range(16):
            encb = f1.tile([P, 4, D], F32, tag="encb")
            nc.sync.dma_start(encb[:], enc_b[bb])
            xh3 = f1.tile([P, 4, D], BF16, tag="xh3")
            ln_norm(f1s, [encb[:, t, :] for t in range(4)], 1e-6, "ln3",
                    [xh3[:, t, :] for t in range(4)])
            xh3T = f1.tile([P, 2, 4 * P], BF16, tag="xh3T")
            for half in range(2):
                tp = psp.tile([P, 2, 2 * P], BF16, tag="ps")
                for tloc in range(2):
                    t = 2 * half + tloc
                    for dc in range(2):
                        nc.tensor.transpose(tp[:, dc, ts(tloc, P)],
                                            xh3[:, t, ts(dc, P)], identb[:])
                for dc in range(2):
                    nc.scalar.copy(xh3T[:, dc, 2 * P * half : 2 * P * (half + 1)],
                                   tp[:, dc, :])
            h1T = f1.tile([P, DI // P, 4 * P], BF16, tag="h1T", bufs=1)
            for ot in range(DI // P):
                hps = psp.tile([P, 4 * P], F32, tag="ps")
                for c in range(2):
                    nc.tensor.matmul(hps[:], w1s[:, c, ts(ot, P)], xh3T[:, c, :],
                                     start=(c == 0), stop=(c == 1))
                nc.scalar.activation(
                    h1T[:, ot, :], hps[:], AF.Gelu, bias=b1f[:, ot, :]
                )
            z = f1.tile([P, 4, D], F32, tag="z")
            for t in range(4):
                yps = psp.tile([P, D], F32, tag="ps")
                for c in range(DI // P):
                    nc.tensor.matmul(yps[:], h1T[:, c, ts(t, P)], w2s[:, c, :],
                                     start=(c == 0), stop=(c == DI // P - 1))
                yb = f1.tile([P, D], F32, tag="yb")
                nc.vector.scalar_tensor_tensor(
                    yb[:], yps[:], 1.0, b2_b[:], op0=ALU.mult, op1=ALU.add
                )
                nc.gpsimd.tensor_tensor(z[:, t, :], yb[:], encb[:, t, :], op=ALU.add)
            xhf = f1.tile([P, 4, D], F32, tag="xhf")
            ln_norm(f1s, [z[:, t, :] for t in range(4)], 1e-6, "lnf",
                    [xhf[:, t, :] for t in range(4)])
            ov = f1.tile([P, 4, D], F32, tag="ov")
            for t in range(4):
                t1 = f1.tile([P, D], F32, tag="t1f")
                nc.vector.tensor_tensor(t1[:], xhf[:, t, :], gf_b[:], op=ALU.mult)
                nc.gpsimd.tensor_tensor(ov[:, t, :], t1[:], bf_b[:], op=ALU.add)
            for i in range(4):
                nc.sync.dma_start(
                    out[:, bb * 16 + i : bb * 16 + i + 13 : 4, :],
                    ov[K * i : K * (i + 1), :, :],
                )

        f1s.release()
        f1.release()
        psp.release()
        wp.release()
        cp.release()

    nc.compile()
    return nc


_NC_CACHE = None


def _get_nc():
    global _NC_CACHE
    if _NC_CACHE is None:
        _NC_CACHE = build_nc()
    return _NC_CACHE


WEIGHT_NAMES = [
    "Wq_t", "Wk_t", "Wv_t", "fcw_t", "fcb_t", "lng_t", "lnb_t",
    "Wq_y", "Wk_y", "Wv_y", "fcw_y", "fcb_y", "lng_y", "lnb_y",
    "w1", "b1", "w2", "b2", "lng", "lnb",
]


def kernel(**inputs):
    from concourse.bass_utils import run_bass_kernel_spmd

    nc = _get_nc()
    x = np.ascontiguousarray(np.asarray(inputs["x"], dtype=np.float32))
    mask = np.ascontiguousarray(np.asarray(inputs["mask"], dtype=np.int32))
    weights = {
        n: np.ascontiguousarray(np.asarray(inputs[n], dtype=np.float32))
        for n in WEIGHT_NAMES
    }
    in_maps = []
    for c in range(B):
        m = {"x": x[c], "mask": mask[c]}
        m.update(weights)
        in_maps.append(m)
    res = run_bass_kernel_spmd(nc, in_maps, core_ids=list(range(B)))
    out = np.stack([res.results[c]["out"] for c in range(B)])
    a_tem = np.stack([res.results[c]["attn_tem"] for c in range(B)])
    a_typ = np.stack([res.results[c]["attn_type"] for c in range(B)])
    return out, a_tem, a_typ


# revision 16
# speedup vs baseline: 18.2302x; 18.2302x over previous
"""Trainium2 Bass kernel for nn_EncoderLayer (dense transformer encoder layer).

Sharding: data-parallel over batch b across 8 NeuronCores (one batch element
per core, weights replicated, no collectives).

Per-core plan:
  - temporal attention per k-slice (L=256 tokens), token-major tiles
    [128 part = l%128, a = l//128, d]
  - tem_out round-trips through DRAM to reorder tokens into "type order"
    t' = l*32 + k; a type tile of 128 tokens = 4 complete l-slices
    (partition p = 32*(l%4) + k)
  - type attention batches 4 l-slices per [128,128] logits tile with a
    block-diagonal mask; fully-masked rows recover uniform 1/K via the
    (S+C)*M01 trick plus a 4x row rescale
  - FFN on 512-token batches in type order; final LN + scatter-store back
    to temporal token order

Matmul operands are kept in bf16 (PE runs 1 cycle/row); PSUM accumulation
and the softmax logits stay fp32.
"""

import sys

for _p in ("/opt/trn_rl_repo",):
    if _p not in sys.path:
        sys.path.append(_p)

import numpy as np
import ml_dtypes

import concourse.bacc as bacc
import concourse.mybir as mybir
import concourse.tile as tile
from concourse.bass import ts

F32 = mybir.dt.float32
BF16 = mybir.dt.bfloat16
I32 = mybir.dt.int32
AF = mybir.ActivationFunctionType
ALU = mybir.AluOpType

B, K, L, D = 8, 32, 256, 256
NH, DK, DV, DI = 4, 64, 64, 1024
P = 128
C_MASK = 64.0


def build_nc():
    nc = bacc.Bacc("TRN2", target_bir_lowering=False, debug=False)

    x = nc.dram_tensor("x", (K, L, D), F32, kind="ExternalInput")
    mask = nc.dram_tensor("mask", (K, L), I32, kind="ExternalInput")
    wq_t = nc.dram_tensor("Wq_t", (D, NH * DK), F32, kind="ExternalInput")
    wk_t = nc.dram_tensor("Wk_t", (D, NH * DK), F32, kind="ExternalInput")
    wv_t = nc.dram_tensor("Wv_t", (D, NH * DV), F32, kind="ExternalInput")
    fcw_t = nc.dram_tensor("fcw_t", (NH * DV, D), F32, kind="ExternalInput")
    fcb_t = nc.dram_tensor("fcb_t", (D,), F32, kind="ExternalInput")
    lng_t = nc.dram_tensor("lng_t", (D,), F32, kind="ExternalInput")
    lnb_t = nc.dram_tensor("lnb_t", (D,), F32, kind="ExternalInput")
    wq_y = nc.dram_tensor("Wq_y", (D, NH * DK), F32, kind="ExternalInput")
    wk_y = nc.dram_tensor("Wk_y", (D, NH * DK), F32, kind="ExternalInput")
    wv_y = nc.dram_tensor("Wv_y", (D, NH * DV), F32, kind="ExternalInput")
    fcw_y = nc.dram_tensor("fcw_y", (NH * DV, D), F32, kind="ExternalInput")
    fcb_y = nc.dram_tensor("fcb_y", (D,), F32, kind="ExternalInput")
    lng_y = nc.dram_tensor("lng_y", (D,), F32, kind="ExternalInput")
    lnb_y = nc.dram_tensor("lnb_y", (D,), F32, kind="ExternalInput")
    w1 = nc.dram_tensor("w1", (D, DI), F32, kind="ExternalInput")
    b1 = nc.dram_tensor("b1", (DI,), F32, kind="ExternalInput")
    w2 = nc.dram_tensor("w2", (DI, D), F32, kind="ExternalInput")
    b2 = nc.dram_tensor("b2", (D,), F32, kind="ExternalInput")
    lng = nc.dram_tensor("lng", (D,), F32, kind="ExternalInput")
    lnb = nc.dram_tensor("lnb", (D,), F32, kind="ExternalInput")

    out = nc.dram_tensor("out", (K, L, D), F32, kind="ExternalOutput")
    attn_tem = nc.dram_tensor("attn_tem", (K, NH, L, L), F32, kind="ExternalOutput")
    attn_type = nc.dram_tensor("attn_type", (L, NH, K, K), F32, kind="ExternalOutput")

    tem_d = nc.dram_tensor("tem_d", (K, L, D), F32, kind="Internal")
    enc_d = nc.dram_tensor("enc_d", (K * L, D), F32, kind="Internal")
    mrow_d = nc.dram_tensor("mrow_d", (K * L,), BF16, kind="Internal")
    mtyp_d = nc.dram_tensor("mtyp_d", (K * L,), BF16, kind="Internal")

    ident_d = nc.inline_tensor(np.eye(P, dtype=np.float32), name="ident_c")
    identb_d = nc.inline_tensor(np.eye(P, dtype=ml_dtypes.bfloat16), name="identb_c")
    onesb_d = nc.inline_tensor(np.ones((1, P), dtype=ml_dtypes.bfloat16), name="onesb_c")
    bdiag_np = np.kron(np.eye(4), np.ones((K, K))).astype(ml_dtypes.bfloat16)
    bdiag_d = nc.inline_tensor(bdiag_np, name="bdiag_c")

    with tile.TileContext(nc) as tc:
        cp = tc.alloc_tile_pool(name="const", bufs=1)
        wp = tc.alloc_tile_pool(name="weights", bufs=1)
        psp = tc.alloc_tile_pool(name="psp", bufs=8, space="PSUM")
        setp = tc.alloc_tile_pool(name="setup", bufs=2)

        ident = cp.tile([P, P], F32)
        nc.sync.dma_start(ident[:], ident_d[:])
        identb = cp.tile([P, P], BF16)
        nc.sync.dma_start(identb[:], identb_d[:])
        onesb = cp.tile([1, P], BF16)
        nc.sync.dma_start(onesb[:], onesb_d[:])
        bdiag = cp.tile([P, P], BF16)
        nc.sync.dma_start(bdiag[:], bdiag_d[:])
        negC = cp.tile([P, 1], F32)
        nc.vector.memset(negC[:], -C_MASK)

        # ---- mask preprocessing ----
        msk_i = setp.tile([K, L], I32)
        nc.sync.dma_start(msk_i[:], mask[:])
        msk_f = setp.tile([K, L], F32)
        nc.vector.tensor_copy(msk_f[:], msk_i[:])
        nc.vector.tensor_scalar(msk_f[:], msk_f[:], 0.0, None, op0=ALU.not_equal)
        msk_b = setp.tile([K, L], BF16)
        nc.vector.tensor_copy(msk_b[:], msk_f[:])
        nc.sync.dma_start(mrow_d.rearrange("(k l) -> k l", k=K), msk_b[:])
        mT = setp.tile([P, 2, K], BF16)
        for a in range(2):
            tps = psp.tile([P, K], F32, tag="ps")
            nc.tensor.transpose(tps[:], msk_f[:, ts(a, P)], ident[0:K, 0:K])
            nc.scalar.copy(mT[:, a, :], tps[:])
        nc.sync.dma_start(mtyp_d.rearrange("(a r k) -> r a k", a=2, r=P), mT[:])

        # ---- weights: load f32, fold shared-LN gain, convert to bf16 ----
        lng_pp = wp.tile([P, 2], F32)
        nc.sync.dma_start(lng_pp[:], lng.rearrange("(c p) -> p c", p=P))
        lnb_ppb = wp.tile([P, 2], BF16)
        lnb_pp = setp.tile([P, 2], F32, tag="lnbpp")
        nc.sync.dma_start(lnb_pp[:], lnb.rearrange("(c p) -> p c", p=P))
        nc.vector.tensor_copy(lnb_ppb[:], lnb_pp[:])

        def load_wb(dram, cdim, odim, name, fold, scale=1.0):
            stage = setp.tile([P, cdim // P, odim], F32, tag="wstage")
            nc.sync.dma_start(stage[:], dram.rearrange("(c p) o -> p c o", p=P))
            t = wp.tile([P, cdim // P, odim], BF16, name=name)
            for c in range(cdim // P):
                if fold:
                    nc.vector.tensor_scalar(
                        stage[:, c, :], stage[:, c, :], lng_pp[:, c : c + 1],
                        float(scale), op0=ALU.mult, op1=ALU.mult,
                    )
                nc.gpsimd.tensor_copy(t[:, c, :], stage[:, c, :])
            return t

        wq_ts = load_wb(wq_t, D, NH * DK, "wq_ts", True, 0.125)
        wk_ts = load_wb(wk_t, D, NH * DK, "wk_ts", True)
        wv_ts = load_wb(wv_t, D, NH * DV, "wv_ts", True)
        fcw_ts = load_wb(fcw_t, NH * DV, D, "fcw_ts", False)
        wq_ys = load_wb(wq_y, D, NH * DK, "wq_ys", True, 0.125)
        wk_ys = load_wb(wk_y, D, NH * DK, "wk_ys", True)
        wv_ys = load_wb(wv_y, D, NH * DV, "wv_ys", True)
        fcw_ys = load_wb(fcw_y, NH * DV, D, "fcw_ys", False)
        w1s = load_wb(w1, D, DI, "w1s", True)
        w2s = load_wb(w2, DI, D, "w2s", False)

        # folded-LN bias per output tile: bW[o] = sum_d lnb[d] * W'[d, o]
        def fold_bias(wt, nout, name):
            bt = wp.tile([P, nout // P, 1], F32, name=name)
            for ot in range(nout // P):
                ps = psp.tile([P, 1], F32, tag="ps")
                for c in range(2):
                    nc.tensor.matmul(ps[:], wt[:, c, ts(ot, P)],
                                     lnb_ppb[:, c : c + 1],
                                     start=(c == 0), stop=(c == 1))
                nc.scalar.copy(bt[:, ot, :], ps[:])
            return bt

        bwq_t = fold_bias(wq_ts, NH * DK, "bwq_t")
        bwk_t = fold_bias(wk_ts, NH * DK, "bwk_t")
        bwq_y = fold_bias(wq_ys, NH * DK, "bwq_y")
        bwk_y = fold_bias(wk_ys, NH * DK, "bwk_y")
        bw1 = fold_bias(w1s, DI, "bw1")

        b1_pp = setp.tile([P, DI // P], F32, tag="b1pp")
        nc.sync.dma_start(b1_pp[:], b1.rearrange("(c p) -> p c", p=P))
        b1f = wp.tile([P, DI // P, 1], F32)
        for c in range(DI // P):
            nc.vector.tensor_tensor(
                b1f[:, c, :], b1_pp[:, c : c + 1], bw1[:, c, :], op=ALU.add
            )

        # broadcast-row tiles [128, D] for per-column vectors
        def bcast_row(dram_vec, name):
            row = setp.tile([1, D], F32, tag="bcrow")
            nc.sync.dma_start(row[:], dram_vec[None, :])
            t = wp.tile([P, D], F32, name=name)
            nc.gpsimd.partition_broadcast(t[:], row[:])
            return t

        fcb_t_b = bcast_row(fcb_t, "fcb_t_b")
        fcb_y_b = bcast_row(fcb_y, "fcb_y_b")
        b2_b = bcast_row(b2, "b2_b")
        gt_b = bcast_row(lng_t, "gt_b")
        bt_b = bcast_row(lnb_t, "bt_b")
        gy_b = bcast_row(lng_y, "gy_b")
        by_b = bcast_row(lnb_y, "by_b")
        gf_b = bcast_row(lng, "gf_b")
        bf_b = bcast_row(lnb, "bf_b")

        # V-bias broadcast rows: bWv[o] broadcast to [128, D]
        def fold_bias_row(wt, name):
            bcol = fold_bias(wt, D, name + "_col")
            rowp = setp.tile([1, D], F32, tag="fbrow")
            for c in range(2):
                tps = psp.tile([1, P], F32, tag="ps")
                nc.tensor.transpose(tps[:], bcol[:, c, :], ident[:])
                nc.scalar.copy(rowp[:, ts(c, P)], tps[:])
            t = wp.tile([P, D], F32, name=name)
            nc.gpsimd.partition_broadcast(t[:], rowp[:])
            return t

        bwv_t_b = fold_bias_row(wv_ts, "bwv_t_b")
        bwv_y_b = fold_bias_row(wv_ys, "bwv_y_b")

        def batched_rstd(pool, var_ap, eps, n, tag):
            veps = pool.tile([P, n], F32, tag=tag + "ve")
            nc.vector.tensor_scalar(veps[:], var_ap, float(eps), None, op0=ALU.add)
            lnv = pool.tile([P, n], F32, tag=tag + "ln")
            nc.scalar.activation(lnv[:], veps[:], AF.Ln)
            rstd = pool.tile([P, n], F32, tag=tag + "rs")
            nc.scalar.activation(rstd[:], lnv[:], AF.Exp, scale=-0.5)
            return rstd

        def ln_norm(pool, src_ap_list, eps, tag, out_tiles):
            """LayerNorm normalize (no gain/bias) for [128, D] APs sharing one
            batched rstd computation."""
            n = len(src_ap_list)
            mv = pool.tile([P, n, 2], F32, tag=tag + "mv")
            for i, ap in enumerate(src_ap_list):
                st = pool.tile([P, 6], F32, tag=tag + "st")
                nc.vector.bn_stats(st[:], ap)
                nc.vector.bn_aggr(mv[:, i, :], st[:])
            rstd = batched_rstd(pool, mv[:, :, 1], eps, n, tag)
            negmr = pool.tile([P, n], F32, tag=tag + "nm")
            nc.vector.scalar_tensor_tensor(
                negmr[:], mv[:, :, 0], -1.0, rstd[:], op0=ALU.mult, op1=ALU.mult
            )
            for i, ap in enumerate(src_ap_list):
                nc.scalar.activation(
                    out_tiles[i], ap, AF.Identity,
                    bias=negmr[:, i : i + 1], scale=rstd[:, i : i + 1],
                )

        # ================= STAGE 1: temporal attention =================
        setp.release()
        s1 = tc.alloc_tile_pool(name="s1", bufs=2)
        s1s = tc.alloc_tile_pool(name="s1_small", bufs=3)

        for k in range(K):
            xk = s1.tile([P, 2, D], F32, tag="xk")
            nc.sync.dma_start(xk[:], x[k].rearrange("(a p) d -> p a d", p=P))

            xh = s1.tile([P, 2, D], BF16, tag="xh")
            ln_norm(s1s, [xk[:, 0, :], xk[:, 1, :]], 1e-6, "ln1",
                    [xh[:, 0, :], xh[:, 1, :]])

            tp = psp.tile([P, 2, 2 * P], BF16, tag="ps")
            for a in range(2):
                for dc in range(2):
                    nc.tensor.transpose(tp[:, dc, ts(a, P)], xh[:, a, ts(dc, P)],
                                        identb[:])
            xhT = s1.tile([P, 2, 2 * P], BF16, tag="xhT")
            nc.scalar.copy(xhT[:], tp[:])

            qT = s1.tile([P, 2, 2 * P], BF16, tag="qT")
            kT = s1.tile([P, 2, 2 * P], BF16, tag="kT")
            for wt, bt, dst in ((wq_ts, bwq_t, qT), (wk_ts, bwk_t, kT)):
                for ot in range(2):
                    ps = psp.tile([P, 2 * P], F32, tag="ps")
                    for c in range(2):
                        nc.tensor.matmul(ps[:], wt[:, c, ts(ot, P)], xhT[:, c, :],
                                         start=(c == 0), stop=(c == 1))
                    nc.scalar.activation(
                        dst[:, ot, :], ps[:], AF.Identity, bias=bt[:, ot, :]
                    )
            v_sb = s1.tile([P, 2, D], BF16, tag="v_sb")
            for a in range(2):
                ps = psp.tile([P, D], F32, tag="ps")
                for c in range(2):
                    nc.tensor.matmul(ps[:], xhT[:, c, ts(a, P)], wv_ts[:, c, :],
                                     start=(c == 0), stop=(c == 1))
                nc.vector.scalar_tensor_tensor(
                    v_sb[:, a, :], ps[:], 1.0, bwv_t_b[:], op0=ALU.mult, op1=ALU.add
                )

            mrow_k = s1s.tile([1, L], BF16, tag="mrowk")
            nc.sync.dma_start(mrow_k[:], mrow_d[None, k * L : (k + 1) * L])
            mps = psp.tile([P, 2, L], F32, tag="ps")
            for lt in range(2):
                nc.tensor.matmul(mps[:, lt, :], mrow_k[:, ts(lt, P)], mrow_k[:],
                                 start=True, stop=True)

            avT = s1.tile([P, 2, 2 * P], BF16, tag="avT")
            avp = psp.tile([P, 2, L], F32, tag="ps")
            for h in range(NH):
                hp = (h % 2) * DK
                q_h = qT[hp : hp + DK, h // 2, :]
                k_h = kT[hp : hp + DK, h // 2, :]
                sps = psp.tile([P, 2, L], F32, tag="ps")
                for lt in range(2):
                    nc.tensor.matmul(sps[:, lt, :], q_h[:, ts(lt, P)], k_h[:],
                                     start=True, stop=True)
                tt = s1.tile([P, 2, L], F32, tag="tt")
                nc.vector.scalar_tensor_tensor(
                    tt[:], sps[:], C_MASK, mps[:], op0=ALU.add, op1=ALU.mult
                )
                atp = psp.tile([P, 2, L], F32, tag="ps")
                den = s1s.tile([P, 2], F32, tag="den")
                rec = s1s.tile([P, 2], F32, tag="rec")
                p_sb = s1.tile([P, 2, L], BF16, tag="p_sb")
                nc.scalar.activation(p_sb[:], tt[:], AF.Exp, bias=negC[:])
                nc.vector.tensor_reduce(
                    den[:], p_sb[:], axis=mybir.AxisListType.X, op=ALU.add
                )
                nc.vector.reciprocal(rec[:], den[:])
                for lt in range(2):
                    attn = s1.tile([P, L], F32, tag="attn")
                    nc.vector.tensor_scalar(
                        attn[:], p_sb[:, lt, :], rec[:, lt : lt + 1], None,
                        op0=ALU.mult
                    )
                    nc.sync.dma_start(attn_tem[k, h, ts(lt, P), :], attn[:])
                    for mt in range(2):
                        nc.tensor.transpose(atp[:, mt, ts(lt, P)],
                                            attn[:, ts(mt, P)], ident[:])
                atT = s1.tile([P, 2, L], BF16, tag="atT")
                nc.scalar.copy(atT[:], atp[:])
                for mt in range(2):
                    nc.tensor.matmul(avp[hp : hp + DV, h // 2, :],
                                     v_sb[:, mt, h * DV : (h + 1) * DV],
                                     atT[:, mt, :],
                                     start=(mt == 0), stop=(mt == 1))
            nc.scalar.copy(avT[:], avp[:])

            tem_sb = s1.tile([P, 2, D], F32, tag="tem_sb")
            ob = s1.tile([P, 2, D], F32, tag="ob")
            for a in range(2):
                ops = psp.tile([P, D], F32, tag="ps")
                for c in range(2):
                    nc.tensor.matmul(ops[:], avT[:, c, ts(a, P)], fcw_ts[:, c, :],
                                     start=(c == 0), stop=(c == 1))
                nc.vector.scalar_tensor_tensor(
                    ob[:, a, :], ops[:], 1.0, fcb_t_b[:], op0=ALU.mult, op1=ALU.add
                )
            xht = s1.tile([P, 2, D], F32, tag="xht")
            ln_norm(s1s, [ob[:, 0, :], ob[:, 1, :]], 1e-5, "lnt",
                    [xht[:, 0, :], xht[:, 1, :]])
            for a in range(2):
                u = s1.tile([P, D], F32, tag="u")
                nc.gpsimd.tensor_tensor(u[:], xk[:, a, :], bt_b[:], op=ALU.add)
                t1 = s1.tile([P, D], F32, tag="t1")
                nc.vector.tensor_tensor(t1[:], xht[:, a, :], gt_b[:], op=ALU.mult)
                nc.gpsimd.tensor_tensor(tem_sb[:, a, :], t1[:], u[:], op=ALU.add)
            nc.sync.dma_start(
                tem_d[k].rearrange("(a p) d -> p a d", p=P), tem_sb[:]
            )

        # ================= STAGE 2: type attention =================
        for gp in range(K):  # 32 pairs of type tiles
            tin = s1.tile([P, 2, D], F32, tag="tin")
            for i in range(4):
                nc.sync.dma_start(
                    tin[K * i : K * (i + 1), :, :],
                    tem_d[:, 8 * gp + i : 8 * gp + i + 5 : 4, :],
                )

            xh2 = s1.tile([P, 2, D], BF16, tag="xh")
            ln_norm(s1s, [tin[:, 0, :], tin[:, 1, :]], 1e-6, "ln2",
                    [xh2[:, 0, :], xh2[:, 1, :]])
            tp = psp.tile([P, 2, 2 * P], BF16, tag="ps")
            for t2 in range(2):
                for dc in range(2):
                    nc.tensor.transpose(tp[:, dc, ts(t2, P)], xh2[:, t2, ts(dc, P)],
                                        identb[:])
            xh2T = s1.tile([P, 2, 2 * P], BF16, tag="xhT")
            nc.scalar.copy(xh2T[:], tp[:])

            q2T = s1.tile([P, 2, 2 * P], BF16, tag="qT")
            k2T = s1.tile([P, 2, 2 * P], BF16, tag="kT")
            for wt, bt, dst in ((wq_ys, bwq_y, q2T), (wk_ys, bwk_y, k2T)):
                for ot in range(2):
                    ps = psp.tile([P, 2 * P], F32, tag="ps")
                    for c in range(2):
                        nc.tensor.matmul(ps[:], wt[:, c, ts(ot, P)], xh2T[:, c, :],
                                         start=(c == 0), stop=(c == 1))
                    nc.scalar.activation(
                        dst[:, ot, :], ps[:], AF.Identity, bias=bt[:, ot, :]
                    )
            v2 = s1.tile([P, 2, D], BF16, tag="v_sb")
            for t2 in range(2):
                ps = psp.tile([P, D], F32, tag="ps")
                for c in range(2):
                    nc.tensor.matmul(ps[:], xh2T[:, c, ts(t2, P)], wv_ys[:, c, :],
                                     start=(c == 0), stop=(c == 1))
                nc.vector.scalar_tensor_tensor(
                    v2[:, t2, :], ps[:], 1.0, bwv_y_b[:], op0=ALU.mult, op1=ALU.add
                )

            enc2 = s1.tile([P, 2, D], F32, tag="enc2")
            ob2 = s1.tile([P, 2, D], F32, tag="ob2")
            for t2 in range(2):
                g = 2 * gp + t2
                vrow_g = s1s.tile([1, P], BF16, tag="vrowg")
                nc.sync.dma_start(vrow_g[:], mtyp_d[None, g * P : (g + 1) * P])
                mps = psp.tile([P, P], F32, tag="ps")
                nc.tensor.matmul(mps[:], vrow_g[:], vrow_g[:], start=True, stop=True)
                m01g = s1.tile([P, P], BF16, tag="m01g")
                nc.vector.tensor_tensor(m01g[:], mps[:], bdiag[:], op=ALU.mult)
                vcp = psp.tile([P, 1], F32, tag="ps")
                nc.tensor.matmul(vcp[:], vrow_g[:], onesb[:, 0:1],
                                 start=True, stop=True)
                rs = s1s.tile([P, 1], F32, tag="rs")
                nc.vector.tensor_scalar(
                    rs[:], vcp[:], -3.0, 4.0, op0=ALU.mult, op1=ALU.add
                )

                attn2_all = s1.tile([P, NH, P], F32, tag="attn2_all")
                av2T = s1.tile([P, 2, P], BF16, tag="av2T")
                av2p = psp.tile([P, 2, P], F32, tag="ps")
                sps_all = psp.tile([P, NH, P], F32, tag="ps")
                for h in range(NH):
                    nc.tensor.matmul(
                        sps_all[:, h, :],
                        q2T[(h % 2) * DK : (h % 2) * DK + DK, h // 2, ts(t2, P)],
                        k2T[(h % 2) * DK : (h % 2) * DK + DK, h // 2, ts(t2, P)],
                        start=True, stop=True)
                tt_all = s1.tile([P, NH, P], F32, tag="tt2")
                m01g_b = m01g.rearrange("p (u m) -> p u m", u=1).broadcast_to([P, NH, P])
                nc.vector.scalar_tensor_tensor(
                    tt_all[:], sps_all[:], C_MASK, m01g_b, op0=ALU.add, op1=ALU.mult
                )
                p_all = s1.tile([P, NH, P], BF16, tag="p_sb2")
                nc.scalar.activation(p_all[:], tt_all[:], AF.Exp, bias=negC[:])
                den = s1s.tile([P, NH], F32, tag="den2")
                nc.vector.tensor_reduce(
                    den[:], p_all[:], axis=mybir.AxisListType.X, op=ALU.add
                )
                rec = s1s.tile([P, NH], F32, tag="rec2a")
                nc.vector.reciprocal(rec[:], den[:])
                rec2 = s1s.tile([P, NH], F32, tag="rec2b")
                nc.vector.tensor_scalar(
                    rec2[:], rec[:], rs[:], None, op0=ALU.mult
                )
                atti = s1.tile([P, NH, P], F32, tag="atti")
                rec2_b = rec2.rearrange("p (h u) -> p h u", u=1).broadcast_to([P, NH, P])
                nc.vector.tensor_tensor(atti[:], p_all[:], rec2_b, op=ALU.mult)
                bdiag_b = bdiag.rearrange("p (u m) -> p u m", u=1).broadcast_to([P, NH, P])
                nc.vector.tensor_tensor(attn2_all[:], atti[:], bdiag_b, op=ALU.mult)
                at2_all = s1.tile([P, NH, P], BF16, tag="at2s")
                nc.vector.transpose(at2_all[:], attn2_all[:])
                for h in range(NH):
                    hp = (h % 2) * DK
                    nc.tensor.matmul(av2p[hp : hp + DV, h // 2, :],
                                     v2[:, t2, h * DV : (h + 1) * DV],
                                     at2_all[:, h, :],
                                     start=True, stop=True)
                for i in range(4):
                    nc.sync.dma_start(
                        attn_type[4 * g + i].rearrange("h k1 k2 -> k1 h k2"),
                        attn2_all[K * i : K * (i + 1), :, K * i : K * (i + 1)],
                    )
                nc.scalar.copy(av2T[:], av2p[:])

                ops = psp.tile([P, D], F32, tag="ps")
                for c in range(2):
                    nc.tensor.matmul(ops[:], av2T[:, c, :], fcw_ys[:, c, :],
                                     start=(c == 0), stop=(c == 1))
                nc.vector.scalar_tensor_tensor(
                    ob2[:, t2, :], ops[:], 1.0, fcb_y_b[:], op0=ALU.mult, op1=ALU.add
                )
            xhy = s1.tile([P, 2, D], F32, tag="xht")
            ln_norm(s1s, [ob2[:, 0, :], ob2[:, 1, :]], 1e-5, "lny",
                    [xhy[:, 0, :], xhy[:, 1, :]])
            for t2 in range(2):
                u = s1.tile([P, D], F32, tag="u")
                nc.gpsimd.tensor_tensor(u[:], tin[:, t2, :], by_b[:], op=ALU.add)
                t1 = s1.tile([P, D], F32, tag="t1")
                nc.vector.tensor_tensor(t1[:], xhy[:, t2, :], gy_b[:], op=ALU.mult)
                nc.gpsimd.tensor_tensor(enc2[:, t2, :], t1[:], u[:], op=ALU.add)
            nc.sync.dma_start(
                enc_d.rearrange("(gg p) d -> gg p d", p=P)[2 * gp], enc2[:, 0, :]
            )
            nc.sync.dma_start(
                enc_d.rearrange("(gg p) d -> gg p d", p=P)[2 * gp + 1], enc2[:, 1, :]
            )

        # ================= STAGE 3: FFN + final LN =================
        s1s.release()
        s1.release()
        f1 = tc.alloc_tile_pool(name="f1", bufs=2)
        f1s = tc.alloc_tile_pool(name="f1_small", bufs=3)
        enc_b = enc_d.rearrange("(b t p) d -> b p t d", t=4, p=P)
        for bb in range(16):
            encb = f1.tile([P, 4, D], F32, tag="encb")
            nc.sync.dma_start(encb[:], enc_b[bb])
            xh3 = f1.tile([P, 4, D], BF16, tag="xh3")
            ln_norm(f1s, [encb[:, t, :] for t in range(4)], 1e-6, "ln3",
                    [xh3[:, t, :] for t in range(4)])
            xh3T = f1.tile([P, 2, 4 * P], BF16, tag="xh3T")
            for half in range(2):
                tp = psp.tile([P, 2, 2 * P], BF16, tag="ps")
                for tloc in range(2):
                    t = 2 * half + tloc
                    for dc in range(2):
                        nc.tensor.transpose(tp[:, dc, ts(tloc, P)],
                                            xh3[:, t, ts(dc, P)], identb[:])
                for dc in range(2):
                    nc.scalar.copy(xh3T[:, dc, 2 * P * half : 2 * P * (half + 1)],
                                   tp[:, dc, :])
            h1T = f1.tile([P, DI // P, 4 * P], BF16, tag="h1T", bufs=1)
            for ot in range(DI // P):
                hps = psp.tile([P, 4 * P], F32, tag="ps")
                for c in range(2):
                    nc.tensor.matmul(hps[:], w1s[:, c, ts(ot, P)], xh3T[:, c, :],
                                     start=(c == 0), stop=(c == 1))
                nc.scalar.activation(
                    h1T[:, ot, :], hps[:], AF.Gelu, bias=b1f[:, ot, :]
                )
            z = f1.tile([P, 4, D], F32, tag="z")
            for t in range(4):
                yps = psp.tile([P, D], F32, tag="ps")
                for c in range(DI // P):
                    nc.tensor.matmul(yps[:], h1T[:, c, ts(t, P)], w2s[:, c, :],
                                     start=(c == 0), stop=(c == DI // P - 1))
                yb = f1.tile([P, D], F32, tag="yb")
                nc.vector.scalar_tensor_tensor(
                    yb[:], yps[:], 1.0, b2_b[:], op0=ALU.mult, op1=ALU.add
                )
                nc.gpsimd.tensor_tensor(z[:, t, :], yb[:], encb[:, t, :], op=ALU.add)
            xhf = f1.tile([P, 4, D], F32, tag="xhf")
            ln_norm(f1s, [z[:, t, :] for t in range(4)], 1e-6, "lnf",
                    [xhf[:, t, :] for t in range(4)])
            ov = f1.tile([P, 4, D], F32, tag="ov")
            for t in range(4):
                t1 = f1.tile([P, D], F32, tag="t1f")
                nc.vector.tensor_tensor(t1[:], xhf[:, t, :], gf_b[:], op=ALU.mult)
                nc.gpsimd.tensor_tensor(ov[:, t, :], t1[:], bf_b[:], op=ALU.add)
            for i in range(4):
                nc.sync.dma_start(
                    out[:, bb * 16 + i : bb * 16 + i + 13 : 4, :],
                    ov[K * i : K * (i + 1), :, :],
                )

        f1s.release()
        f1.release()
        psp.release()
        wp.release()
        cp.release()

    nc.compile()
    return nc


_NC_CACHE = None


def _get_nc():
    global _NC_CACHE
    if _NC_CACHE is None:
        _NC_CACHE = build_nc()
    return _NC_CACHE


WEIGHT_NAMES = [
    "Wq_t", "Wk_t", "Wv_t", "fcw_t", "fcb_t", "lng_t", "lnb_t",
    "Wq_y", "Wk_y", "Wv_y", "fcw_y", "fcb_y", "lng_y", "lnb_y",
    "w1", "b1", "w2", "b2", "lng", "lnb",
]


def kernel(**inputs):
    from concourse.bass_utils import run_bass_kernel_spmd

    nc = _get_nc()
    x = np.ascontiguousarray(np.asarray(inputs["x"], dtype=np.float32))
    mask = np.ascontiguousarray(np.asarray(inputs["mask"], dtype=np.int32))
    weights = {
        n: np.ascontiguousarray(np.asarray(inputs[n], dtype=np.float32))
        for n in WEIGHT_NAMES
    }
    in_maps = []
    for c in range(B):
        m = {"x": x[c], "mask": mask[c]}
        m.update(weights)
        in_maps.append(m)
    res = run_bass_kernel_spmd(nc, in_maps, core_ids=list(range(B)))
    out = np.stack([res.results[c]["out"] for c in range(B)])
    a_tem = np.stack([res.results[c]["attn_tem"] for c in range(B)])
    a_typ = np.stack([res.results[c]["attn_type"] for c in range(B)])
    return out, a_tem, a_typ


# revision 19
# speedup vs baseline: 18.5758x; 1.0190x over previous
"""Trainium2 Bass kernel for nn_EncoderLayer (dense transformer encoder layer).

Sharding: data-parallel over batch b across 8 NeuronCores (one batch element
per core, weights replicated, no collectives).

Per-core plan:
  - temporal attention per k-slice (L=256 tokens), token-major tiles
    [128 part = l%128, a = l//128, d]
  - tem_out round-trips through DRAM to reorder tokens into "type order"
    t' = l*32 + k; a type tile of 128 tokens = 4 complete l-slices
    (partition p = 32*(l%4) + k)
  - type attention batches 4 l-slices per [128,128] logits tile with a
    block-diagonal mask; fully-masked rows recover uniform 1/K via the
    (S+C)*M01 trick plus a 4x row rescale
  - FFN on 512-token batches in type order; final LN + scatter-store back
    to temporal token order

Matmul operands are kept in bf16 (PE runs 1 cycle/row); PSUM accumulation
and the softmax logits stay fp32.
"""

import sys

for _p in ("/opt/trn_rl_repo",):
    if _p not in sys.path:
        sys.path.append(_p)

import numpy as np
import ml_dtypes

import concourse.bacc as bacc
import concourse.mybir as mybir
import concourse.tile as tile
from concourse.bass import ts

F32 = mybir.dt.float32
BF16 = mybir.dt.bfloat16
I32 = mybir.dt.int32
AF = mybir.ActivationFunctionType
ALU = mybir.AluOpType

B, K, L, D = 8, 32, 256, 256
NH, DK, DV, DI = 4, 64, 64, 1024
P = 128
C_MASK = 64.0


def build_nc():
    nc = bacc.Bacc("TRN2", target_bir_lowering=False, debug=False)

    x = nc.dram_tensor("x", (K, L, D), F32, kind="ExternalInput")
    mask = nc.dram_tensor("mask", (K, L), I32, kind="ExternalInput")
    wq_t = nc.dram_tensor("Wq_t", (D, NH * DK), F32, kind="ExternalInput")
    wk_t = nc.dram_tensor("Wk_t", (D, NH * DK), F32, kind="ExternalInput")
    wv_t = nc.dram_tensor("Wv_t", (D, NH * DV), F32, kind="ExternalInput")
    fcw_t = nc.dram_tensor("fcw_t", (NH * DV, D), F32, kind="ExternalInput")
    fcb_t = nc.dram_tensor("fcb_t", (D,), F32, kind="ExternalInput")
    lng_t = nc.dram_tensor("lng_t", (D,), F32, kind="ExternalInput")
    lnb_t = nc.dram_tensor("lnb_t", (D,), F32, kind="ExternalInput")
    wq_y = nc.dram_tensor("Wq_y", (D, NH * DK), F32, kind="ExternalInput")
    wk_y = nc.dram_tensor("Wk_y", (D, NH * DK), F32, kind="ExternalInput")
    wv_y = nc.dram_tensor("Wv_y", (D, NH * DV), F32, kind="ExternalInput")
    fcw_y = nc.dram_tensor("fcw_y", (NH * DV, D), F32, kind="ExternalInput")
    fcb_y = nc.dram_tensor("fcb_y", (D,), F32, kind="ExternalInput")
    lng_y = nc.dram_tensor("lng_y", (D,), F32, kind="ExternalInput")
    lnb_y = nc.dram_tensor("lnb_y", (D,), F32, kind="ExternalInput")
    w1 = nc.dram_tensor("w1", (D, DI), F32, kind="ExternalInput")
    b1 = nc.dram_tensor("b1", (DI,), F32, kind="ExternalInput")
    w2 = nc.dram_tensor("w2", (DI, D), F32, kind="ExternalInput")
    b2 = nc.dram_tensor("b2", (D,), F32, kind="ExternalInput")
    lng = nc.dram_tensor("lng", (D,), F32, kind="ExternalInput")
    lnb = nc.dram_tensor("lnb", (D,), F32, kind="ExternalInput")

    out = nc.dram_tensor("out", (K, L, D), F32, kind="ExternalOutput")
    attn_tem = nc.dram_tensor("attn_tem", (K, NH, L, L), F32, kind="ExternalOutput")
    attn_type = nc.dram_tensor("attn_type", (L, NH, K, K), F32, kind="ExternalOutput")

    tem_d = nc.dram_tensor("tem_d", (K, L, D), F32, kind="Internal")
    enc_d = nc.dram_tensor("enc_d", (K * L, D), F32, kind="Internal")
    mrow_d = nc.dram_tensor("mrow_d", (K * L,), BF16, kind="Internal")
    mtyp_d = nc.dram_tensor("mtyp_d", (K * L,), BF16, kind="Internal")

    ident_d = nc.inline_tensor(np.eye(P, dtype=np.float32), name="ident_c")
    identb_d = nc.inline_tensor(np.eye(P, dtype=ml_dtypes.bfloat16), name="identb_c")
    onesb_d = nc.inline_tensor(np.ones((1, P), dtype=ml_dtypes.bfloat16), name="onesb_c")
    bdiag_np = np.kron(np.eye(4), np.ones((K, K))).astype(ml_dtypes.bfloat16)
    bdiag_d = nc.inline_tensor(bdiag_np, name="bdiag_c")

    with tile.TileContext(nc) as tc:
        cp = tc.alloc_tile_pool(name="const", bufs=1)
        wp = tc.alloc_tile_pool(name="weights", bufs=1)
        psp = tc.alloc_tile_pool(name="psp", bufs=8, space="PSUM")
        setp = tc.alloc_tile_pool(name="setup", bufs=2)

        ident = cp.tile([P, P], F32)
        nc.sync.dma_start(ident[:], ident_d[:])
        identb = cp.tile([P, P], BF16)
        nc.sync.dma_start(identb[:], identb_d[:])
        onesb = cp.tile([1, P], BF16)
        nc.sync.dma_start(onesb[:], onesb_d[:])
        bdiag = cp.tile([P, P], BF16)
        nc.sync.dma_start(bdiag[:], bdiag_d[:])
        negC = cp.tile([P, 1], F32)
        nc.vector.memset(negC[:], -C_MASK)

        # ---- mask preprocessing ----
        msk_i = setp.tile([K, L], I32)
        nc.sync.dma_start(msk_i[:], mask[:])
        msk_f = setp.tile([K, L], F32)
        nc.vector.tensor_copy(msk_f[:], msk_i[:])
        nc.vector.tensor_scalar(msk_f[:], msk_f[:], 0.0, None, op0=ALU.not_equal)
        msk_b = setp.tile([K, L], BF16)
        nc.vector.tensor_copy(msk_b[:], msk_f[:])
        nc.sync.dma_start(mrow_d.rearrange("(k l) -> k l", k=K), msk_b[:])
        mT = setp.tile([P, 2, K], BF16)
        for a in range(2):
            tps = psp.tile([P, K], F32, tag="ps")
            nc.tensor.transpose(tps[:], msk_f[:, ts(a, P)], ident[0:K, 0:K])
            nc.scalar.copy(mT[:, a, :], tps[:])
        nc.sync.dma_start(mtyp_d.rearrange("(a r k) -> r a k", a=2, r=P), mT[:])

        # ---- weights: load f32, fold shared-LN gain, convert to bf16 ----
        lng_pp = wp.tile([P, 2], F32)
        nc.sync.dma_start(lng_pp[:], lng.rearrange("(c p) -> p c", p=P))
        lnb_ppb = wp.tile([P, 2], BF16)
        lnb_pp = setp.tile([P, 2], F32, tag="lnbpp")
        nc.sync.dma_start(lnb_pp[:], lnb.rearrange("(c p) -> p c", p=P))
        nc.vector.tensor_copy(lnb_ppb[:], lnb_pp[:])

        def load_wb(dram, cdim, odim, name, fold, scale=1.0):
            stage = setp.tile([P, cdim // P, odim], F32, tag="wstage")
            nc.sync.dma_start(stage[:], dram.rearrange("(c p) o -> p c o", p=P))
            t = wp.tile([P, cdim // P, odim], BF16, name=name)
            for c in range(cdim // P):
                if fold:
                    nc.vector.tensor_scalar(
                        stage[:, c, :], stage[:, c, :], lng_pp[:, c : c + 1],
                        float(scale), op0=ALU.mult, op1=ALU.mult,
                    )
                nc.gpsimd.tensor_copy(t[:, c, :], stage[:, c, :])
            return t

        wq_ts = load_wb(wq_t, D, NH * DK, "wq_ts", True, 0.125)
        wk_ts = load_wb(wk_t, D, NH * DK, "wk_ts", True)
        wv_ts = load_wb(wv_t, D, NH * DV, "wv_ts", True)
        fcw_ts = load_wb(fcw_t, NH * DV, D, "fcw_ts", False)
        wq_ys = load_wb(wq_y, D, NH * DK, "wq_ys", True, 0.125)
        wk_ys = load_wb(wk_y, D, NH * DK, "wk_ys", True)
        wv_ys = load_wb(wv_y, D, NH * DV, "wv_ys", True)
        fcw_ys = load_wb(fcw_y, NH * DV, D, "fcw_ys", False)
        w1s = load_wb(w1, D, DI, "w1s", True)
        w2s = load_wb(w2, DI, D, "w2s", False)

        # folded-LN bias per output tile: bW[o] = sum_d lnb[d] * W'[d, o]
        def fold_bias(wt, nout, name):
            bt = wp.tile([P, nout // P, 1], F32, name=name)
            for ot in range(nout // P):
                ps = psp.tile([P, 1], F32, tag="ps")
                for c in range(2):
                    nc.tensor.matmul(ps[:], wt[:, c, ts(ot, P)],
                                     lnb_ppb[:, c : c + 1],
                                     start=(c == 0), stop=(c == 1))
                nc.scalar.copy(bt[:, ot, :], ps[:])
            return bt

        bwq_t = fold_bias(wq_ts, NH * DK, "bwq_t")
        bwk_t = fold_bias(wk_ts, NH * DK, "bwk_t")
        bwq_y = fold_bias(wq_ys, NH * DK, "bwq_y")
        bwk_y = fold_bias(wk_ys, NH * DK, "bwk_y")
        bw1 = fold_bias(w1s, DI, "bw1")

        b1_pp = setp.tile([P, DI // P], F32, tag="b1pp")
        nc.sync.dma_start(b1_pp[:], b1.rearrange("(c p) -> p c", p=P))
        b1f = wp.tile([P, DI // P, 1], F32)
        for c in range(DI // P):
            nc.vector.tensor_tensor(
                b1f[:, c, :], b1_pp[:, c : c + 1], bw1[:, c, :], op=ALU.add
            )

        # broadcast-row tiles [128, D] for per-column vectors
        def bcast_row(dram_vec, name):
            row = setp.tile([1, D], F32, tag="bcrow")
            nc.sync.dma_start(row[:], dram_vec[None, :])
            t = wp.tile([P, D], F32, name=name)
            nc.gpsimd.partition_broadcast(t[:], row[:])
            return t

        fcb_t_b = bcast_row(fcb_t, "fcb_t_b")
        fcb_y_b = bcast_row(fcb_y, "fcb_y_b")
        b2_b = bcast_row(b2, "b2_b")
        gt_b = bcast_row(lng_t, "gt_b")
        bt_b = bcast_row(lnb_t, "bt_b")
        gy_b = bcast_row(lng_y, "gy_b")
        by_b = bcast_row(lnb_y, "by_b")
        gf_b = bcast_row(lng, "gf_b")
        bf_b = bcast_row(lnb, "bf_b")

        # V-bias broadcast rows: bWv[o] broadcast to [128, D]
        def fold_bias_row(wt, name):
            bcol = fold_bias(wt, D, name + "_col")
            rowp = setp.tile([1, D], F32, tag="fbrow")
            for c in range(2):
                tps = psp.tile([1, P], F32, tag="ps")
                nc.tensor.transpose(tps[:], bcol[:, c, :], ident[:])
                nc.scalar.copy(rowp[:, ts(c, P)], tps[:])
            t = wp.tile([P, D], F32, name=name)
            nc.gpsimd.partition_broadcast(t[:], rowp[:])
            return t

        bwv_t_b = fold_bias_row(wv_ts, "bwv_t_b")
        bwv_y_b = fold_bias_row(wv_ys, "bwv_y_b")

        def batched_rstd(pool, var_ap, eps, n, tag):
            veps = pool.tile([P, n], F32, tag=tag + "ve")
            nc.vector.tensor_scalar(veps[:], var_ap, float(eps), None, op0=ALU.add)
            lnv = pool.tile([P, n], F32, tag=tag + "ln")
            nc.scalar.activation(lnv[:], veps[:], AF.Ln)
            rstd = pool.tile([P, n], F32, tag=tag + "rs")
            nc.scalar.activation(rstd[:], lnv[:], AF.Exp, scale=-0.5)
            return rstd

        def ln_norm(pool, src_ap_list, eps, tag, out_tiles):
            """LayerNorm normalize (no gain/bias) for [128, D] APs sharing one
            batched rstd computation."""
            n = len(src_ap_list)
            mv = pool.tile([P, n, 2], F32, tag=tag + "mv")
            for i, ap in enumerate(src_ap_list):
                st = pool.tile([P, 6], F32, tag=tag + "st")
                nc.vector.bn_stats(st[:], ap)
                nc.vector.bn_aggr(mv[:, i, :], st[:])
            rstd = batched_rstd(pool, mv[:, :, 1], eps, n, tag)
            negmr = pool.tile([P, n], F32, tag=tag + "nm")
            nc.vector.scalar_tensor_tensor(
                negmr[:], mv[:, :, 0], -1.0, rstd[:], op0=ALU.mult, op1=ALU.mult
            )
            for i, ap in enumerate(src_ap_list):
                nc.scalar.activation(
                    out_tiles[i], ap, AF.Identity,
                    bias=negmr[:, i : i + 1], scale=rstd[:, i : i + 1],
                )

        # ================= STAGE 1: temporal attention =================
        setp.release()
        s1 = tc.alloc_tile_pool(name="s1", bufs=2)
        s1s = tc.alloc_tile_pool(name="s1_small", bufs=3)

        for k in range(K):
            xk = s1.tile([P, 2, D], F32, tag="xk")
            nc.sync.dma_start(xk[:], x[k].rearrange("(a p) d -> p a d", p=P))

            xh = s1.tile([P, 2, D], BF16, tag="xh")
            ln_norm(s1s, [xk[:, 0, :], xk[:, 1, :]], 1e-6, "ln1",
                    [xh[:, 0, :], xh[:, 1, :]])

            tp = psp.tile([P, 2, 2 * P], BF16, tag="ps")
            for a in range(2):
                for dc in range(2):
                    nc.tensor.transpose(tp[:, dc, ts(a, P)], xh[:, a, ts(dc, P)],
                                        identb[:])
            xhT = s1.tile([P, 2, 2 * P], BF16, tag="xhT")
            nc.scalar.copy(xhT[:], tp[:])

            qT = s1.tile([P, 2, 2 * P], BF16, tag="qT")
            kT = s1.tile([P, 2, 2 * P], BF16, tag="kT")
            for wt, bt, dst in ((wq_ts, bwq_t, qT), (wk_ts, bwk_t, kT)):
                for ot in range(2):
                    ps = psp.tile([P, 2 * P], F32, tag="ps")
                    for c in range(2):
                        nc.tensor.matmul(ps[:], wt[:, c, ts(ot, P)], xhT[:, c, :],
                                         start=(c == 0), stop=(c == 1))
                    nc.scalar.activation(
                        dst[:, ot, :], ps[:], AF.Identity, bias=bt[:, ot, :]
                    )
            v_sb = s1.tile([P, 2, D], BF16, tag="v_sb")
            for a in range(2):
                ps = psp.tile([P, D], F32, tag="ps")
                for c in range(2):
                    nc.tensor.matmul(ps[:], xhT[:, c, ts(a, P)], wv_ts[:, c, :],
                                     start=(c == 0), stop=(c == 1))
                nc.vector.scalar_tensor_tensor(
                    v_sb[:, a, :], ps[:], 1.0, bwv_t_b[:], op0=ALU.mult, op1=ALU.add
                )

            mrow_k = s1s.tile([1, L], BF16, tag="mrowk")
            nc.sync.dma_start(mrow_k[:], mrow_d[None, k * L : (k + 1) * L])
            mps = psp.tile([P, 2, L], F32, tag="ps")
            for lt in range(2):
                nc.tensor.matmul(mps[:, lt, :], mrow_k[:, ts(lt, P)], mrow_k[:],
                                 start=True, stop=True)
            m01 = s1.tile([P, 2, L], BF16, tag="m01")
            nc.scalar.copy(m01[:], mps[:])

            avT = s1.tile([P, 2, 2 * P], BF16, tag="avT")
            avp = psp.tile([P, 2, L], F32, tag="ps")
            for h in range(NH):
                hp = (h % 2) * DK
                q_h = qT[hp : hp + DK, h // 2, :]
                k_h = kT[hp : hp + DK, h // 2, :]
                sps = psp.tile([P, 2, L], F32, tag="ps")
                for lt in range(2):
                    nc.tensor.matmul(sps[:, lt, :], q_h[:, ts(lt, P)], k_h[:],
                                     start=True, stop=True)
                tt = s1.tile([P, 2, L], F32, tag="tt")
                nc.vector.scalar_tensor_tensor(
                    tt[:], sps[:], C_MASK, m01[:], op0=ALU.add, op1=ALU.mult
                )
                atp = psp.tile([P, 2, L], F32, tag="ps")
                den = s1s.tile([P, 2], F32, tag="den")
                rec = s1s.tile([P, 2], F32, tag="rec")
                for lt in range(2):
                    p_sb = s1.tile([P, L], BF16, tag="p_sb")
                    nc.scalar.activation(
                        p_sb[:], tt[:, lt, :], AF.Exp, bias=negC[:],
                        accum_out=den[:, lt : lt + 1],
                    )
                    nc.vector.reciprocal(rec[:, lt : lt + 1], den[:, lt : lt + 1])
                    attn = s1.tile([P, L], F32, tag="attn")
                    nc.vector.tensor_scalar(
                        attn[:], p_sb[:], rec[:, lt : lt + 1], None, op0=ALU.mult
                    )
                    nc.sync.dma_start(attn_tem[k, h, ts(lt, P), :], attn[:])
                    for mt in range(2):
                        nc.tensor.transpose(atp[:, mt, ts(lt, P)],
                                            attn[:, ts(mt, P)], ident[:])
                atT = s1.tile([P, 2, L], BF16, tag="atT")
                nc.scalar.copy(atT[:], atp[:])
                for mt in range(2):
                    nc.tensor.matmul(avp[hp : hp + DV, h // 2, :],
                                     v_sb[:, mt, h * DV : (h + 1) * DV],
                                     atT[:, mt, :],
                                     start=(mt == 0), stop=(mt == 1))
            nc.scalar.copy(avT[:], avp[:])

            tem_sb = s1.tile([P, 2, D], F32, tag="tem_sb")
            ob = s1.tile([P, 2, D], F32, tag="ob")
            for a in range(2):
                ops = psp.tile([P, D], F32, tag="ps")
                for c in range(2):
                    nc.tensor.matmul(ops[:], avT[:, c, ts(a, P)], fcw_ts[:, c, :],
                                     start=(c == 0), stop=(c == 1))
                nc.vector.scalar_tensor_tensor(
                    ob[:, a, :], ops[:], 1.0, fcb_t_b[:], op0=ALU.mult, op1=ALU.add
                )
            xht = s1.tile([P, 2, D], F32, tag="xht")
            ln_norm(s1s, [ob[:, 0, :], ob[:, 1, :]], 1e-5, "lnt",
                    [xht[:, 0, :], xht[:, 1, :]])
            for a in range(2):
                u = s1.tile([P, D], F32, tag="u")
                nc.gpsimd.tensor_tensor(u[:], xk[:, a, :], bt_b[:], op=ALU.add)
                t1 = s1.tile([P, D], F32, tag="t1")
                nc.vector.tensor_tensor(t1[:], xht[:, a, :], gt_b[:], op=ALU.mult)
                nc.gpsimd.tensor_tensor(tem_sb[:, a, :], t1[:], u[:], op=ALU.add)
            nc.sync.dma_start(
                tem_d[k].rearrange("(a p) d -> p a d", p=P), tem_sb[:]
            )

        # ================= STAGE 2: type attention =================
        for gp in range(K):  # 32 pairs of type tiles
            tin = s1.tile([P, 2, D], F32, tag="tin")
            for i in range(4):
                nc.sync.dma_start(
                    tin[K * i : K * (i + 1), :, :],
                    tem_d[:, 8 * gp + i : 8 * gp + i + 5 : 4, :],
                )

            xh2 = s1.tile([P, 2, D], BF16, tag="xh")
            ln_norm(s1s, [tin[:, 0, :], tin[:, 1, :]], 1e-6, "ln2",
                    [xh2[:, 0, :], xh2[:, 1, :]])
            tp = psp.tile([P, 2, 2 * P], BF16, tag="ps")
            for t2 in range(2):
                for dc in range(2):
                    nc.tensor.transpose(tp[:, dc, ts(t2, P)], xh2[:, t2, ts(dc, P)],
                                        identb[:])
            xh2T = s1.tile([P, 2, 2 * P], BF16, tag="xhT")
            nc.scalar.copy(xh2T[:], tp[:])

            q2T = s1.tile([P, 2, 2 * P], BF16, tag="qT")
            k2T = s1.tile([P, 2, 2 * P], BF16, tag="kT")
            for wt, bt, dst in ((wq_ys, bwq_y, q2T), (wk_ys, bwk_y, k2T)):
                for ot in range(2):
                    ps = psp.tile([P, 2 * P], F32, tag="ps")
                    for c in range(2):
                        nc.tensor.matmul(ps[:], wt[:, c, ts(ot, P)], xh2T[:, c, :],
                                         start=(c == 0), stop=(c == 1))
                    nc.scalar.activation(
                        dst[:, ot, :], ps[:], AF.Identity, bias=bt[:, ot, :]
                    )
            v2 = s1.tile([P, 2, D], BF16, tag="v_sb")
            for t2 in range(2):
                ps = psp.tile([P, D], F32, tag="ps")
                for c in range(2):
                    nc.tensor.matmul(ps[:], xh2T[:, c, ts(t2, P)], wv_ys[:, c, :],
                                     start=(c == 0), stop=(c == 1))
                nc.vector.scalar_tensor_tensor(
                    v2[:, t2, :], ps[:], 1.0, bwv_y_b[:], op0=ALU.mult, op1=ALU.add
                )

            enc2 = s1.tile([P, 2, D], F32, tag="enc2")
            ob2 = s1.tile([P, 2, D], F32, tag="ob2")
            for t2 in range(2):
                g = 2 * gp + t2
                vrow_g = s1s.tile([1, P], BF16, tag="vrowg")
                nc.sync.dma_start(vrow_g[:], mtyp_d[None, g * P : (g + 1) * P])
                mps = psp.tile([P, P], F32, tag="ps")
                nc.tensor.matmul(mps[:], vrow_g[:], vrow_g[:], start=True, stop=True)
                m01g = s1.tile([P, P], BF16, tag="m01g")
                nc.vector.tensor_tensor(m01g[:], mps[:], bdiag[:], op=ALU.mult)
                vcp = psp.tile([P, 1], F32, tag="ps")
                nc.tensor.matmul(vcp[:], vrow_g[:], onesb[:, 0:1],
                                 start=True, stop=True)
                rs = s1s.tile([P, 1], F32, tag="rs")
                nc.vector.tensor_scalar(
                    rs[:], vcp[:], -3.0, 4.0, op0=ALU.mult, op1=ALU.add
                )

                attn2_all = s1.tile([P, NH, P], F32, tag="attn2_all")
                av2T = s1.tile([P, 2, P], BF16, tag="av2T")
                av2p = psp.tile([P, 2, P], F32, tag="ps")
                for h in range(NH):
                    hp = (h % 2) * DK
                    sps = psp.tile([P, P], F32, tag="ps")
                    nc.tensor.matmul(sps[:], q2T[hp : hp + DK, h // 2, ts(t2, P)],
                                     k2T[hp : hp + DK, h // 2, ts(t2, P)],
                                     start=True, stop=True)
                    tt = s1.tile([P, P], F32, tag="tt2")
                    nc.vector.scalar_tensor_tensor(
                        tt[:], sps[:], C_MASK, m01g[:], op0=ALU.add, op1=ALU.mult
                    )
                    den2 = s1s.tile([P, 1], F32, tag="den2")
                    p_sb = s1.tile([P, P], BF16, tag="p_sb2")
                    nc.scalar.activation(
                        p_sb[:], tt[:], AF.Exp, bias=negC[:], accum_out=den2[:]
                    )
                    rec = s1s.tile([P, 1], F32, tag="rec2a")
                    nc.vector.reciprocal(rec[:], den2[:])
                    rec2 = s1s.tile([P, 1], F32, tag="rec2b")
                    nc.vector.tensor_tensor(rec2[:], rec[:], rs[:], op=ALU.mult)
                    nc.vector.scalar_tensor_tensor(
                        attn2_all[:, h, :], p_sb[:], rec2[:], bdiag[:],
                        op0=ALU.mult, op1=ALU.mult,
                    )
                    atp = psp.tile([P, P], F32, tag="ps")
                    nc.tensor.transpose(atp[:], attn2_all[:, h, :], ident[:])
                    at2 = s1.tile([P, P], BF16, tag="at2s")
                    nc.scalar.copy(at2[:], atp[:])
                    nc.tensor.matmul(av2p[hp : hp + DV, h // 2, :],
                                     v2[:, t2, h * DV : (h + 1) * DV], at2[:],
                                     start=True, stop=True)
                for i in range(4):
                    nc.sync.dma_start(
                        attn_type[4 * g + i].rearrange("h k1 k2 -> k1 h k2"),
                        attn2_all[K * i : K * (i + 1), :, K * i : K * (i + 1)],
                    )
                nc.scalar.copy(av2T[:], av2p[:])

                ops = psp.tile([P, D], F32, tag="ps")
                for c in range(2):
                    nc.tensor.matmul(ops[:], av2T[:, c, :], fcw_ys[:, c, :],
                                     start=(c == 0), stop=(c == 1))
                nc.vector.scalar_tensor_tensor(
                    ob2[:, t2, :], ops[:], 1.0, fcb_y_b[:], op0=ALU.mult, op1=ALU.add
                )
            xhy = s1.tile([P, 2, D], F32, tag="xht")
            ln_norm(s1s, [ob2[:, 0, :], ob2[:, 1, :]], 1e-5, "lny",
                    [xhy[:, 0, :], xhy[:, 1, :]])
            for t2 in range(2):
                u = s1.tile([P, D], F32, tag="u")
                nc.gpsimd.tensor_tensor(u[:], tin[:, t2, :], by_b[:], op=ALU.add)
                t1 = s1.tile([P, D], F32, tag="t1")
                nc.vector.tensor_tensor(t1[:], xhy[:, t2, :], gy_b[:], op=ALU.mult)
                nc.gpsimd.tensor_tensor(enc2[:, t2, :], t1[:], u[:], op=ALU.add)
            nc.sync.dma_start(
                enc_d.rearrange("(gg p) d -> gg p d", p=P)[2 * gp], enc2[:, 0, :]
            )
            nc.sync.dma_start(
                enc_d.rearrange("(gg p) d -> gg p d", p=P)[2 * gp + 1], enc2[:, 1, :]
            )

        # ================= STAGE 3: FFN + final LN =================
        s1s.release()
        s1.release()
        f1 = tc.alloc_tile_pool(name="f1", bufs=2)
        f1s = tc.alloc_tile_pool(name="f1_small", bufs=3)
        enc_b = enc_d.rearrange("(b t p) d -> b p t d", t=4, p=P)
        for bb in range(16):
            encb = f1.tile([P, 4, D], F32, tag="encb")
            nc.sync.dma_start(encb[:], enc_b[bb])
            xh3 = f1.tile([P, 4, D], BF16, tag="xh3")
            ln_norm(f1s, [encb[:, t, :] for t in range(4)], 1e-6, "ln3",
                    [xh3[:, t, :] for t in range(4)])
            xh3T = f1.tile([P, 2, 4 * P], BF16, tag="xh3T")
            for half in range(2):
                tp = psp.tile([P, 2, 2 * P], BF16, tag="ps")
                for tloc in range(2):
                    t = 2 * half + tloc
                    for dc in range(2):
                        nc.tensor.transpose(tp[:, dc, ts(tloc, P)],
                                            xh3[:, t, ts(dc, P)], identb[:])
                for dc in range(2):
                    nc.scalar.copy(xh3T[:, dc, 2 * P * half : 2 * P * (half + 1)],
                                   tp[:, dc, :])
            h1T = f1.tile([P, DI // P, 4 * P], BF16, tag="h1T", bufs=1)
            for ot in range(DI // P):
                hps = psp.tile([P, 4 * P], F32, tag="ps")
                for c in range(2):
                    nc.tensor.matmul(hps[:], w1s[:, c, ts(ot, P)], xh3T[:, c, :],
                                     start=(c == 0), stop=(c == 1))
                nc.scalar.activation(
                    h1T[:, ot, :], hps[:], AF.Gelu, bias=b1f[:, ot, :]
                )
            z = f1.tile([P, 4, D], F32, tag="z")
            for t in range(4):
                yps = psp.tile([P, D], F32, tag="ps")
                for c in range(DI // P):
                    nc.tensor.matmul(yps[:], h1T[:, c, ts(t, P)], w2s[:, c, :],
                                     start=(c == 0), stop=(c == DI // P - 1))
                yb = f1.tile([P, D], F32, tag="yb")
                nc.vector.scalar_tensor_tensor(
                    yb[:], yps[:], 1.0, b2_b[:], op0=ALU.mult, op1=ALU.add
                )
                nc.gpsimd.tensor_tensor(z[:, t, :], yb[:], encb[:, t, :], op=ALU.add)
            xhf = f1.tile([P, 4, D], F32, tag="xhf")
            ln_norm(f1s, [z[:, t, :] for t in range(4)], 1e-6, "lnf",
                    [xhf[:, t, :] for t in range(4)])
            ov = f1.tile([P, 4, D], F32, tag="ov")
            for t in range(4):
                t1 = f1.tile([P, D], F32, tag="t1f")
                nc.vector.tensor_tensor(t1[:], xhf[:, t, :], gf_b[:], op=ALU.mult)
                nc.gpsimd.tensor_tensor(ov[:, t, :], t1[:], bf_b[:], op=ALU.add)
            for i in range(4):
                nc.sync.dma_start(
                    out[:, bb * 16 + i : bb * 16 + i + 13 : 4, :],
                    ov[K * i : K * (i + 1), :, :],
                )

        f1s.release()
        f1.release()
        psp.release()
        wp.release()
        cp.release()

    nc.compile()
    return nc


_NC_CACHE = None


def _get_nc():
    global _NC_CACHE
    if _NC_CACHE is None:
        _NC_CACHE = build_nc()
    return _NC_CACHE


WEIGHT_NAMES = [
    "Wq_t", "Wk_t", "Wv_t", "fcw_t", "fcb_t", "lng_t", "lnb_t",
    "Wq_y", "Wk_y", "Wv_y", "fcw_y", "fcb_y", "lng_y", "lnb_y",
    "w1", "b1", "w2", "b2", "lng", "lnb",
]


def kernel(**inputs):
    from concourse.bass_utils import run_bass_kernel_spmd

    nc = _get_nc()
    x = np.ascontiguousarray(np.asarray(inputs["x"], dtype=np.float32))
    mask = np.ascontiguousarray(np.asarray(inputs["mask"], dtype=np.int32))
    weights = {
        n: np.ascontiguousarray(np.asarray(inputs[n], dtype=np.float32))
        for n in WEIGHT_NAMES
    }
    in_maps = []
    for c in range(B):
        m = {"x": x[c], "mask": mask[c]}
        m.update(weights)
        in_maps.append(m)
    res = run_bass_kernel_spmd(nc, in_maps, core_ids=list(range(B)))
    out = np.stack([res.results[c]["out"] for c in range(B)])
    a_tem = np.stack([res.results[c]["attn_tem"] for c in range(B)])
    a_typ = np.stack([res.results[c]["attn_type"] for c in range(B)])
    return out, a_tem, a_typ


# revision 21
# speedup vs baseline: 21.1837x; 1.1404x over previous
"""Trainium2 Bass kernel for nn_EncoderLayer (dense transformer encoder layer).

Sharding: data-parallel over batch b across 8 NeuronCores (one batch element
per core, weights replicated, no collectives).

Per-core plan:
  - temporal attention per k-slice (L=256 tokens), token-major tiles
    [128 part = l%128, a = l//128, d]
  - tem_out round-trips through DRAM to reorder tokens into "type order"
    t' = l*32 + k; a type tile of 128 tokens = 4 complete l-slices
    (partition p = 32*(l%4) + k)
  - type attention batches 4 l-slices per [128,128] logits tile with a
    block-diagonal mask; fully-masked rows recover uniform 1/K via the
    (S+C)*M01 trick plus a 4x row rescale
  - FFN on 512-token batches in type order; final LN + scatter-store back
    to temporal token order

Matmul operands are kept in bf16 (PE runs 1 cycle/row); PSUM accumulation
and the softmax logits stay fp32.
"""

import sys

for _p in ("/opt/trn_rl_repo",):
    if _p not in sys.path:
        sys.path.append(_p)

import numpy as np
import ml_dtypes

import concourse.bacc as bacc
import concourse.mybir as mybir
import concourse.tile as tile
from concourse.bass import ts

F32 = mybir.dt.float32
BF16 = mybir.dt.bfloat16
I32 = mybir.dt.int32
AF = mybir.ActivationFunctionType
ALU = mybir.AluOpType

B, K, L, D = 8, 32, 256, 256
NH, DK, DV, DI = 4, 64, 64, 1024
P = 128
C_MASK = 64.0


def build_nc():
    nc = bacc.Bacc("TRN2", target_bir_lowering=False, debug=False)

    x = nc.dram_tensor("x", (K, L, D), F32, kind="ExternalInput")
    mask = nc.dram_tensor("mask", (K, L), I32, kind="ExternalInput")
    wq_t = nc.dram_tensor("Wq_t", (D, NH * DK), F32, kind="ExternalInput")
    wk_t = nc.dram_tensor("Wk_t", (D, NH * DK), F32, kind="ExternalInput")
    wv_t = nc.dram_tensor("Wv_t", (D, NH * DV), F32, kind="ExternalInput")
    fcw_t = nc.dram_tensor("fcw_t", (NH * DV, D), F32, kind="ExternalInput")
    fcb_t = nc.dram_tensor("fcb_t", (D,), F32, kind="ExternalInput")
    lng_t = nc.dram_tensor("lng_t", (D,), F32, kind="ExternalInput")
    lnb_t = nc.dram_tensor("lnb_t", (D,), F32, kind="ExternalInput")
    wq_y = nc.dram_tensor("Wq_y", (D, NH * DK), F32, kind="ExternalInput")
    wk_y = nc.dram_tensor("Wk_y", (D, NH * DK), F32, kind="ExternalInput")
    wv_y = nc.dram_tensor("Wv_y", (D, NH * DV), F32, kind="ExternalInput")
    fcw_y = nc.dram_tensor("fcw_y", (NH * DV, D), F32, kind="ExternalInput")
    fcb_y = nc.dram_tensor("fcb_y", (D,), F32, kind="ExternalInput")
    lng_y = nc.dram_tensor("lng_y", (D,), F32, kind="ExternalInput")
    lnb_y = nc.dram_tensor("lnb_y", (D,), F32, kind="ExternalInput")
    w1 = nc.dram_tensor("w1", (D, DI), F32, kind="ExternalInput")
    b1 = nc.dram_tensor("b1", (DI,), F32, kind="ExternalInput")
    w2 = nc.dram_tensor("w2", (DI, D), F32, kind="ExternalInput")
    b2 = nc.dram_tensor("b2", (D,), F32, kind="ExternalInput")
    lng = nc.dram_tensor("lng", (D,), F32, kind="ExternalInput")
    lnb = nc.dram_tensor("lnb", (D,), F32, kind="ExternalInput")

    out = nc.dram_tensor("out", (K, L, D), F32, kind="ExternalOutput")
    attn_tem = nc.dram_tensor("attn_tem", (K, NH, L, L), F32, kind="ExternalOutput")
    attn_type = nc.dram_tensor("attn_type", (L, NH, K, K), F32, kind="ExternalOutput")

    tem_d = nc.dram_tensor("tem_d", (K, L, D), F32, kind="Internal")
    enc_d = nc.dram_tensor("enc_d", (K * L, D), F32, kind="Internal")
    mrow_d = nc.dram_tensor("mrow_d", (K * L,), BF16, kind="Internal")
    mtyp_d = nc.dram_tensor("mtyp_d", (K * L,), BF16, kind="Internal")

    ident_d = nc.inline_tensor(np.eye(P, dtype=np.float32), name="ident_c")
    identb_d = nc.inline_tensor(np.eye(P, dtype=ml_dtypes.bfloat16), name="identb_c")
    onesb_d = nc.inline_tensor(np.ones((1, P), dtype=ml_dtypes.bfloat16), name="onesb_c")
    bdiag_np = np.kron(np.eye(4), np.ones((K, K))).astype(ml_dtypes.bfloat16)
    bdiag_d = nc.inline_tensor(bdiag_np, name="bdiag_c")

    with tile.TileContext(nc) as tc:
        cp = tc.alloc_tile_pool(name="const", bufs=1)
        wp = tc.alloc_tile_pool(name="weights", bufs=1)
        psp = tc.alloc_tile_pool(name="psp", bufs=8, space="PSUM")
        setp = tc.alloc_tile_pool(name="setup", bufs=2)

        ident = cp.tile([P, P], F32)
        nc.sync.dma_start(ident[:], ident_d[:])
        identb = cp.tile([P, P], BF16)
        nc.sync.dma_start(identb[:], identb_d[:])
        onesb = cp.tile([1, P], BF16)
        nc.sync.dma_start(onesb[:], onesb_d[:])
        bdiag = cp.tile([P, P], BF16)
        nc.sync.dma_start(bdiag[:], bdiag_d[:])
        negC = cp.tile([P, 1], F32)
        nc.vector.memset(negC[:], -C_MASK)

        # ---- mask preprocessing ----
        msk_i = setp.tile([K, L], I32)
        nc.sync.dma_start(msk_i[:], mask[:])
        msk_f = setp.tile([K, L], F32)
        nc.vector.tensor_copy(msk_f[:], msk_i[:])
        nc.vector.tensor_scalar(msk_f[:], msk_f[:], 0.0, None, op0=ALU.not_equal)
        msk_b = setp.tile([K, L], BF16)
        nc.vector.tensor_copy(msk_b[:], msk_f[:])
        nc.sync.dma_start(mrow_d.rearrange("(k l) -> k l", k=K), msk_b[:])
        mT = setp.tile([P, 2, K], BF16)
        for a in range(2):
            tps = psp.tile([P, K], F32, tag="ps")
            nc.tensor.transpose(tps[:], msk_f[:, ts(a, P)], ident[0:K, 0:K])
            nc.scalar.copy(mT[:, a, :], tps[:])
        nc.sync.dma_start(mtyp_d.rearrange("(a r k) -> r a k", a=2, r=P), mT[:])

        # ---- weights: load f32, fold shared-LN gain, convert to bf16 ----
        lng_pp = wp.tile([P, 2], F32)
        nc.sync.dma_start(lng_pp[:], lng.rearrange("(c p) -> p c", p=P))
        lnb_ppb = wp.tile([P, 2], BF16)
        lnb_pp = setp.tile([P, 2], F32, tag="lnbpp")
        nc.sync.dma_start(lnb_pp[:], lnb.rearrange("(c p) -> p c", p=P))
        nc.vector.tensor_copy(lnb_ppb[:], lnb_pp[:])

        def load_wb(dram, cdim, odim, name, fold, scale=1.0):
            stage = setp.tile([P, cdim // P, odim], F32, tag="wstage")
            nc.sync.dma_start(stage[:], dram.rearrange("(c p) o -> p c o", p=P))
            t = wp.tile([P, cdim // P, odim], BF16, name=name)
            for c in range(cdim // P):
                if fold:
                    nc.vector.tensor_scalar(
                        stage[:, c, :], stage[:, c, :], lng_pp[:, c : c + 1],
                        float(scale), op0=ALU.mult, op1=ALU.mult,
                    )
                nc.gpsimd.tensor_copy(t[:, c, :], stage[:, c, :])
            return t

        wq_ts = load_wb(wq_t, D, NH * DK, "wq_ts", True, 0.125)
        wk_ts = load_wb(wk_t, D, NH * DK, "wk_ts", True)
        wv_ts = load_wb(wv_t, D, NH * DV, "wv_ts", True)
        fcw_ts = load_wb(fcw_t, NH * DV, D, "fcw_ts", False)
        wq_ys = load_wb(wq_y, D, NH * DK, "wq_ys", True, 0.125)
        wk_ys = load_wb(wk_y, D, NH * DK, "wk_ys", True)
        wv_ys = load_wb(wv_y, D, NH * DV, "wv_ys", True)
        fcw_ys = load_wb(fcw_y, NH * DV, D, "fcw_ys", False)
        w1s = load_wb(w1, D, DI, "w1s", True)
        w2s = load_wb(w2, DI, D, "w2s", False)

        # folded-LN bias per output tile: bW[o] = sum_d lnb[d] * W'[d, o]
        def fold_bias(wt, nout, name):
            bt = wp.tile([P, nout // P, 1], F32, name=name)
            for ot in range(nout // P):
                ps = psp.tile([P, 1], F32, tag="ps")
                for c in range(2):
                    nc.tensor.matmul(ps[:], wt[:, c, ts(ot, P)],
                                     lnb_ppb[:, c : c + 1],
                                     start=(c == 0), stop=(c == 1))
                nc.scalar.copy(bt[:, ot, :], ps[:])
            return bt

        bwq_t = fold_bias(wq_ts, NH * DK, "bwq_t")
        bwk_t = fold_bias(wk_ts, NH * DK, "bwk_t")
        bwq_y = fold_bias(wq_ys, NH * DK, "bwq_y")
        bwk_y = fold_bias(wk_ys, NH * DK, "bwk_y")
        bw1 = fold_bias(w1s, DI, "bw1")

        b1_pp = setp.tile([P, DI // P], F32, tag="b1pp")
        nc.sync.dma_start(b1_pp[:], b1.rearrange("(c p) -> p c", p=P))
        b1f = wp.tile([P, DI // P, 1], F32)
        for c in range(DI // P):
            nc.vector.tensor_tensor(
                b1f[:, c, :], b1_pp[:, c : c + 1], bw1[:, c, :], op=ALU.add
            )

        # broadcast-row tiles [128, D] for per-column vectors
        def bcast_row(dram_vec, name):
            row = setp.tile([1, D], F32, tag="bcrow")
            nc.sync.dma_start(row[:], dram_vec[None, :])
            t = wp.tile([P, D], F32, name=name)
            nc.gpsimd.partition_broadcast(t[:], row[:])
            return t

        fcb_t_b = bcast_row(fcb_t, "fcb_t_b")
        fcb_y_b = bcast_row(fcb_y, "fcb_y_b")
        b2_b = bcast_row(b2, "b2_b")
        gt_b = bcast_row(lng_t, "gt_b")
        bt_b = bcast_row(lnb_t, "bt_b")
        gy_b = bcast_row(lng_y, "gy_b")
        by_b = bcast_row(lnb_y, "by_b")
        gf_b = bcast_row(lng, "gf_b")
        bf_b = bcast_row(lnb, "bf_b")

        # V-bias broadcast rows: bWv[o] broadcast to [128, D]
        def fold_bias_row(wt, name):
            bcol = fold_bias(wt, D, name + "_col")
            rowp = setp.tile([1, D], F32, tag="fbrow")
            for c in range(2):
                tps = psp.tile([1, P], F32, tag="ps")
                nc.tensor.transpose(tps[:], bcol[:, c, :], ident[:])
                nc.scalar.copy(rowp[:, ts(c, P)], tps[:])
            t = wp.tile([P, D], F32, name=name)
            nc.gpsimd.partition_broadcast(t[:], rowp[:])
            return t

        bwv_t_b = fold_bias_row(wv_ts, "bwv_t_b")
        bwv_y_b = fold_bias_row(wv_ys, "bwv_y_b")

        def batched_rstd(pool, var_ap, eps, n, tag):
            veps = pool.tile([P, n], F32, tag=tag + "ve")
            nc.vector.tensor_scalar(veps[:], var_ap, float(eps), None, op0=ALU.add)
            lnv = pool.tile([P, n], F32, tag=tag + "ln")
            nc.scalar.activation(lnv[:], veps[:], AF.Ln)
            rstd = pool.tile([P, n], F32, tag=tag + "rs")
            nc.scalar.activation(rstd[:], lnv[:], AF.Exp, scale=-0.5)
            return rstd

        def ln_norm(pool, src_ap_list, eps, tag, out_tiles):
            """LayerNorm normalize (no gain/bias) for [128, D] APs sharing one
            batched rstd computation."""
            n = len(src_ap_list)
            mv = pool.tile([P, n, 2], F32, tag=tag + "mv")
            for i, ap in enumerate(src_ap_list):
                st = pool.tile([P, 6], F32, tag=tag + "st")
                nc.vector.bn_stats(st[:], ap)
                nc.vector.bn_aggr(mv[:, i, :], st[:])
            rstd = batched_rstd(pool, mv[:, :, 1], eps, n, tag)
            negmr = pool.tile([P, n], F32, tag=tag + "nm")
            nc.vector.scalar_tensor_tensor(
                negmr[:], mv[:, :, 0], -1.0, rstd[:], op0=ALU.mult, op1=ALU.mult
            )
            for i, ap in enumerate(src_ap_list):
                nc.vector.tensor_scalar(
                    out_tiles[i], ap, rstd[:, i : i + 1], negmr[:, i : i + 1],
                    op0=ALU.mult, op1=ALU.add,
                )

        # ================= STAGE 1: temporal attention =================
        setp.release()
        s1 = tc.alloc_tile_pool(name="s1", bufs=3)
        s1s = tc.alloc_tile_pool(name="s1_small", bufs=4)

        for k in range(K):
            xk = s1.tile([P, 2, D], F32, tag="xk")
            nc.sync.dma_start(xk[:], x[k].rearrange("(a p) d -> p a d", p=P))

            xh = s1.tile([P, 2, D], BF16, tag="xh")
            ln_norm(s1s, [xk[:, 0, :], xk[:, 1, :]], 1e-6, "ln1",
                    [xh[:, 0, :], xh[:, 1, :]])

            tp = psp.tile([P, 2, 2 * P], BF16, tag="ps")
            for a in range(2):
                for dc in range(2):
                    nc.tensor.transpose(tp[:, dc, ts(a, P)], xh[:, a, ts(dc, P)],
                                        identb[:])
            xhT = s1.tile([P, 2, 2 * P], BF16, tag="xhT")
            nc.scalar.copy(xhT[:], tp[:])

            qT = s1.tile([P, 2, 2 * P], BF16, tag="qT")
            kT = s1.tile([P, 2, 2 * P], BF16, tag="kT")
            for wt, bt, dst in ((wq_ts, bwq_t, qT), (wk_ts, bwk_t, kT)):
                for ot in range(2):
                    ps = psp.tile([P, 2 * P], F32, tag="ps")
                    for c in range(2):
                        nc.tensor.matmul(ps[:], wt[:, c, ts(ot, P)], xhT[:, c, :],
                                         start=(c == 0), stop=(c == 1))
                    nc.vector.tensor_scalar(
                        dst[:, ot, :], ps[:], bt[:, ot, :], None, op0=ALU.add
                    )
            v_sb = s1.tile([P, 2, D], BF16, tag="v_sb")
            for a in range(2):
                ps = psp.tile([P, D], F32, tag="ps")
                for c in range(2):
                    nc.tensor.matmul(ps[:], xhT[:, c, ts(a, P)], wv_ts[:, c, :],
                                     start=(c == 0), stop=(c == 1))
                nc.vector.scalar_tensor_tensor(
                    v_sb[:, a, :], ps[:], 1.0, bwv_t_b[:], op0=ALU.mult, op1=ALU.add
                )

            mrow_k = s1s.tile([1, L], BF16, tag="mrowk")
            nc.sync.dma_start(mrow_k[:], mrow_d[None, k * L : (k + 1) * L])
            mps = psp.tile([P, 2, L], F32, tag="ps")
            for lt in range(2):
                nc.tensor.matmul(mps[:, lt, :], mrow_k[:, ts(lt, P)], mrow_k[:],
                                 start=True, stop=True)
            m01 = s1.tile([P, 2, L], BF16, tag="m01")
            nc.vector.tensor_copy(m01[:], mps[:])

            avT = s1.tile([P, 2, 2 * P], BF16, tag="avT")
            avp = psp.tile([P, 2, L], F32, tag="ps")
            for h in range(NH):
                hp = (h % 2) * DK
                q_h = qT[hp : hp + DK, h // 2, :]
                k_h = kT[hp : hp + DK, h // 2, :]
                sps = psp.tile([P, 2, L], F32, tag="ps")
                for lt in range(2):
                    nc.tensor.matmul(sps[:, lt, :], q_h[:, ts(lt, P)], k_h[:],
                                     start=True, stop=True)
                tt = s1.tile([P, 2, L], F32, tag="tt")
                nc.vector.scalar_tensor_tensor(
                    tt[:], sps[:], C_MASK, m01[:], op0=ALU.add, op1=ALU.mult
                )
                atp = psp.tile([P, 2, L], F32, tag="ps")
                den = s1s.tile([P, 2], F32, tag="den")
                rec = s1s.tile([P, 2], F32, tag="rec")
                for lt in range(2):
                    p_sb = s1.tile([P, L], BF16, tag="p_sb")
                    nc.scalar.activation(
                        p_sb[:], tt[:, lt, :], AF.Exp, bias=negC[:],
                        accum_out=den[:, lt : lt + 1],
                    )
                    nc.vector.reciprocal(rec[:, lt : lt + 1], den[:, lt : lt + 1])
                    attn = s1.tile([P, L], F32, tag="attn")
                    nc.vector.tensor_scalar(
                        attn[:], p_sb[:], rec[:, lt : lt + 1], None, op0=ALU.mult
                    )
                    nc.sync.dma_start(attn_tem[k, h, ts(lt, P), :], attn[:])
                    for mt in range(2):
                        nc.tensor.transpose(atp[:, mt, ts(lt, P)],
                                            attn[:, ts(mt, P)], ident[:])
                atT = s1.tile([P, 2, L], BF16, tag="atT")
                nc.scalar.copy(atT[:], atp[:])
                for mt in range(2):
                    nc.tensor.matmul(avp[hp : hp + DV, h // 2, :],
                                     v_sb[:, mt, h * DV : (h + 1) * DV],
                                     atT[:, mt, :],
                                     start=(mt == 0), stop=(mt == 1))
            nc.vector.tensor_copy(avT[:], avp[:])

            tem_sb = s1.tile([P, 2, D], F32, tag="tem_sb")
            ob = s1.tile([P, 2, D], F32, tag="ob")
            for a in range(2):
                ops = psp.tile([P, D], F32, tag="ps")
                for c in range(2):
                    nc.tensor.matmul(ops[:], avT[:, c, ts(a, P)], fcw_ts[:, c, :],
                                     start=(c == 0), stop=(c == 1))
                nc.vector.scalar_tensor_tensor(
                    ob[:, a, :], ops[:], 1.0, fcb_t_b[:], op0=ALU.mult, op1=ALU.add
                )
            xht = s1.tile([P, 2, D], F32, tag="xht")
            ln_norm(s1s, [ob[:, 0, :], ob[:, 1, :]], 1e-5, "lnt",
                    [xht[:, 0, :], xht[:, 1, :]])
            for a in range(2):
                u = s1.tile([P, D], F32, tag="u")
                nc.gpsimd.tensor_tensor(u[:], xk[:, a, :], bt_b[:], op=ALU.add)
                t1 = s1.tile([P, D], F32, tag="t1")
                nc.vector.tensor_tensor(t1[:], xht[:, a, :], gt_b[:], op=ALU.mult)
                nc.gpsimd.tensor_tensor(tem_sb[:, a, :], t1[:], u[:], op=ALU.add)
            nc.sync.dma_start(
                tem_d[k].rearrange("(a p) d -> p a d", p=P), tem_sb[:]
            )

        # ================= STAGE 2: type attention =================
        for gp in range(K):  # 32 pairs of type tiles
            tin = s1.tile([P, 2, D], F32, tag="tin")
            for i in range(4):
                nc.sync.dma_start(
                    tin[K * i : K * (i + 1), :, :],
                    tem_d[:, 8 * gp + i : 8 * gp + i + 5 : 4, :],
                )

            xh2 = s1.tile([P, 2, D], BF16, tag="xh")
            ln_norm(s1s, [tin[:, 0, :], tin[:, 1, :]], 1e-6, "ln2",
                    [xh2[:, 0, :], xh2[:, 1, :]])
            tp = psp.tile([P, 2, 2 * P], BF16, tag="ps")
            for t2 in range(2):
                for dc in range(2):
                    nc.tensor.transpose(tp[:, dc, ts(t2, P)], xh2[:, t2, ts(dc, P)],
                                        identb[:])
            xh2T = s1.tile([P, 2, 2 * P], BF16, tag="xhT")
            nc.scalar.copy(xh2T[:], tp[:])

            q2T = s1.tile([P, 2, 2 * P], BF16, tag="qT")
            k2T = s1.tile([P, 2, 2 * P], BF16, tag="kT")
            for wt, bt, dst in ((wq_ys, bwq_y, q2T), (wk_ys, bwk_y, k2T)):
                for ot in range(2):
                    ps = psp.tile([P, 2 * P], F32, tag="ps")
                    for c in range(2):
                        nc.tensor.matmul(ps[:], wt[:, c, ts(ot, P)], xh2T[:, c, :],
                                         start=(c == 0), stop=(c == 1))
                    nc.vector.tensor_scalar(
                        dst[:, ot, :], ps[:], bt[:, ot, :], None, op0=ALU.add
                    )
            v2 = s1.tile([P, 2, D], BF16, tag="v_sb")
            for t2 in range(2):
                ps = psp.tile([P, D], F32, tag="ps")
                for c in range(2):
                    nc.tensor.matmul(ps[:], xh2T[:, c, ts(t2, P)], wv_ys[:, c, :],
                                     start=(c == 0), stop=(c == 1))
                nc.vector.scalar_tensor_tensor(
                    v2[:, t2, :], ps[:], 1.0, bwv_y_b[:], op0=ALU.mult, op1=ALU.add
                )

            enc2 = s1.tile([P, 2, D], F32, tag="enc2")
            ob2 = s1.tile([P, 2, D], F32, tag="ob2")
            for t2 in range(2):
                g = 2 * gp + t2
                vrow_g = s1s.tile([1, P], BF16, tag="vrowg")
                nc.sync.dma_start(vrow_g[:], mtyp_d[None, g * P : (g + 1) * P])
                mps = psp.tile([P, P], F32, tag="ps")
                nc.tensor.matmul(mps[:], vrow_g[:], vrow_g[:], start=True, stop=True)
                m01g = s1.tile([P, P], BF16, tag="m01g")
                nc.vector.tensor_tensor(m01g[:], mps[:], bdiag[:], op=ALU.mult)
                vcp = psp.tile([P, 1], F32, tag="ps")
                nc.tensor.matmul(vcp[:], vrow_g[:], onesb[:, 0:1],
                                 start=True, stop=True)
                rs = s1s.tile([P, 1], F32, tag="rs")
                nc.vector.tensor_scalar(
                    rs[:], vcp[:], -3.0, 4.0, op0=ALU.mult, op1=ALU.add
                )

                attn2_all = s1.tile([P, NH, P], F32, tag="attn2_all")
                av2T = s1.tile([P, 2, P], BF16, tag="av2T")
                av2p = psp.tile([P, 2, P], F32, tag="ps")
                for h in range(NH):
                    hp = (h % 2) * DK
                    sps = psp.tile([P, P], F32, tag="ps")
                    nc.tensor.matmul(sps[:], q2T[hp : hp + DK, h // 2, ts(t2, P)],
                                     k2T[hp : hp + DK, h // 2, ts(t2, P)],
                                     start=True, stop=True)
                    tt = s1.tile([P, P], F32, tag="tt2")
                    nc.vector.scalar_tensor_tensor(
                        tt[:], sps[:], C_MASK, m01g[:], op0=ALU.add, op1=ALU.mult
                    )
                    den2 = s1s.tile([P, 1], F32, tag="den2")
                    p_sb = s1.tile([P, P], BF16, tag="p_sb2")
                    nc.scalar.activation(
                        p_sb[:], tt[:], AF.Exp, bias=negC[:], accum_out=den2[:]
                    )
                    rec = s1s.tile([P, 1], F32, tag="rec2a")
                    nc.vector.reciprocal(rec[:], den2[:])
                    rec2 = s1s.tile([P, 1], F32, tag="rec2b")
                    nc.vector.tensor_tensor(rec2[:], rec[:], rs[:], op=ALU.mult)
                    nc.vector.scalar_tensor_tensor(
                        attn2_all[:, h, :], p_sb[:], rec2[:], bdiag[:],
                        op0=ALU.mult, op1=ALU.mult,
                    )
                    atp = psp.tile([P, P], F32, tag="ps")
                    nc.tensor.transpose(atp[:], attn2_all[:, h, :], ident[:])
                    at2 = s1.tile([P, P], BF16, tag="at2s")
                    nc.scalar.copy(at2[:], atp[:])
                    nc.tensor.matmul(av2p[hp : hp + DV, h // 2, :],
                                     v2[:, t2, h * DV : (h + 1) * DV], at2[:],
                                     start=True, stop=True)
                for i in range(4):
                    nc.sync.dma_start(
                        attn_type[4 * g + i].rearrange("h k1 k2 -> k1 h k2"),
                        attn2_all[K * i : K * (i + 1), :, K * i : K * (i + 1)],
                    )
                nc.vector.tensor_copy(av2T[:], av2p[:])

                ops = psp.tile([P, D], F32, tag="ps")
                for c in range(2):
                    nc.tensor.matmul(ops[:], av2T[:, c, :], fcw_ys[:, c, :],
                                     start=(c == 0), stop=(c == 1))
                nc.vector.scalar_tensor_tensor(
                    ob2[:, t2, :], ops[:], 1.0, fcb_y_b[:], op0=ALU.mult, op1=ALU.add
                )
            xhy = s1.tile([P, 2, D], F32, tag="xht")
            ln_norm(s1s, [ob2[:, 0, :], ob2[:, 1, :]], 1e-5, "lny",
                    [xhy[:, 0, :], xhy[:, 1, :]])
            for t2 in range(2):
                u = s1.tile([P, D], F32, tag="u")
                nc.gpsimd.tensor_tensor(u[:], tin[:, t2, :], by_b[:], op=ALU.add)
                t1 = s1.tile([P, D], F32, tag="t1")
                nc.vector.tensor_tensor(t1[:], xhy[:, t2, :], gy_b[:], op=ALU.mult)
                nc.gpsimd.tensor_tensor(enc2[:, t2, :], t1[:], u[:], op=ALU.add)
            nc.sync.dma_start(
                enc_d.rearrange("(gg p) d -> gg p d", p=P)[2 * gp], enc2[:, 0, :]
            )
            nc.sync.dma_start(
                enc_d.rearrange("(gg p) d -> gg p d", p=P)[2 * gp + 1], enc2[:, 1, :]
            )

        # ================= STAGE 3: FFN + final LN =================
        s1s.release()
        s1.release()
        f1 = tc.alloc_tile_pool(name="f1", bufs=3)
        f1s = tc.alloc_tile_pool(name="f1_small", bufs=3)
        enc_b = enc_d.rearrange("(b t p) d -> b p t d", t=4, p=P)
        for bb in range(16):
            encb = f1.tile([P, 4, D], F32, tag="encb")
            nc.sync.dma_start(encb[:], enc_b[bb])
            xh3 = f1.tile([P, 4, D], BF16, tag="xh3")
            ln_norm(f1s, [encb[:, t, :] for t in range(4)], 1e-6, "ln3",
                    [xh3[:, t, :] for t in range(4)])
            xh3T = f1.tile([P, 2, 4 * P], BF16, tag="xh3T")
            for half in range(2):
                tp = psp.tile([P, 2, 2 * P], BF16, tag="ps")
                for tloc in range(2):
                    t = 2 * half + tloc
                    for dc in range(2):
                        nc.tensor.transpose(tp[:, dc, ts(tloc, P)],
                                            xh3[:, t, ts(dc, P)], identb[:])
                for dc in range(2):
                    nc.scalar.copy(xh3T[:, dc, 2 * P * half : 2 * P * (half + 1)],
                                   tp[:, dc, :])
            h1T = f1.tile([P, DI // P, 4 * P], BF16, tag="h1T", bufs=2)
            for ot in range(DI // P):
                hps = psp.tile([P, 4 * P], F32, tag="ps")
                for c in range(2):
                    nc.tensor.matmul(hps[:], w1s[:, c, ts(ot, P)], xh3T[:, c, :],
                                     start=(c == 0), stop=(c == 1))
                nc.scalar.activation(
                    h1T[:, ot, :], hps[:], AF.Gelu, bias=b1f[:, ot, :]
                )
            z = f1.tile([P, 4, D], F32, tag="z")
            for t in range(4):
                yps = psp.tile([P, D], F32, tag="ps")
                for c in range(DI // P):
                    nc.tensor.matmul(yps[:], h1T[:, c, ts(t, P)], w2s[:, c, :],
                                     start=(c == 0), stop=(c == DI // P - 1))
                yb = f1.tile([P, D], F32, tag="yb")
                nc.vector.scalar_tensor_tensor(
                    yb[:], yps[:], 1.0, b2_b[:], op0=ALU.mult, op1=ALU.add
                )
                nc.gpsimd.tensor_tensor(z[:, t, :], yb[:], encb[:, t, :], op=ALU.add)
            xhf = f1.tile([P, 4, D], F32, tag="xhf")
            ln_norm(f1s, [z[:, t, :] for t in range(4)], 1e-6, "lnf",
                    [xhf[:, t, :] for t in range(4)])
            ov = f1.tile([P, 4, D], F32, tag="ov")
            for t in range(4):
                t1 = f1.tile([P, D], F32, tag="t1f")
                nc.vector.tensor_tensor(t1[:], xhf[:, t, :], gf_b[:], op=ALU.mult)
                nc.gpsimd.tensor_tensor(ov[:, t, :], t1[:], bf_b[:], op=ALU.add)
            for i in range(4):
                nc.sync.dma_start(
                    out[:, bb * 16 + i : bb * 16 + i + 13 : 4, :],
                    ov[K * i : K * (i + 1), :, :],
                )

        f1s.release()
        f1.release()
        psp.release()
        wp.release()
        cp.release()

    nc.compile()
    return nc


_NC_CACHE = None


def _get_nc():
    global _NC_CACHE
    if _NC_CACHE is None:
        _NC_CACHE = build_nc()
    return _NC_CACHE


WEIGHT_NAMES = [
    "Wq_t", "Wk_t", "Wv_t", "fcw_t", "fcb_t", "lng_t", "lnb_t",
    "Wq_y", "Wk_y", "Wv_y", "fcw_y", "fcb_y", "lng_y", "lnb_y",
    "w1", "b1", "w2", "b2", "lng", "lnb",
]


def kernel(**inputs):
    from concourse.bass_utils import run_bass_kernel_spmd

    nc = _get_nc()
    x = np.ascontiguousarray(np.asarray(inputs["x"], dtype=np.float32))
    mask = np.ascontiguousarray(np.asarray(inputs["mask"], dtype=np.int32))
    weights = {
        n: np.ascontiguousarray(np.asarray(inputs[n], dtype=np.float32))
        for n in WEIGHT_NAMES
    }
    in_maps = []
    for c in range(B):
        m = {"x": x[c], "mask": mask[c]}
        m.update(weights)
        in_maps.append(m)
    res = run_bass_kernel_spmd(nc, in_maps, core_ids=list(range(B)))
    out = np.stack([res.results[c]["out"] for c in range(B)])
    a_tem = np.stack([res.results[c]["attn_tem"] for c in range(B)])
    a_typ = np.stack([res.results[c]["attn_type"] for c in range(B)])
    return out, a_tem, a_typ
